# revision 6
# baseline (speedup 1.0000x reference)
"""Trainium2 Bass kernel for nn_Branch_Cell (branched LSTM-style cell).

Sharding: expert parallelism over the naxis dimension (naxis == 8 == n_cores).
Core `a` owns axis `a`: it streams that axis's ~189 MB of weights from HBM
(the memory roofline), computes h_m[a], c_m[a] and the partial sums
t1[a], t2[a]; the host sums t1/t2 over axes and applies the final
sigmoid*tanh (8 KB of work, avoids the collective latency floor).

Weight mode 'bf16x2': each fp32 weight matrix is decomposed on the host into
bf16 hi + bf16 lo (same total bytes as fp32). The PE streams both halves at
full bf16 rate with the input vector's (hi, lo) pair as a [K,2] stationary
operand, accumulating all four cross terms in fp32 PSUM:
  (x_hi+x_lo) @ (W_hi+W_lo)  ~  x @ W  to ~1e-6 relative.
Mode 'f32' streams fp32 weights directly (4 cyc/row on PE, slightly slower
than HBM rate).

All matvecs keep the vector as the PE stationary operand (the weights
stream through as the moving operand), so kernel time is bounded by
HBM->SBUF DMA. Elementwise gate/LN math runs on single-partition rows
(ACT + DVE); row->partition-chunk conversion for downstream lhsT operands
is done with tiny K=1 transpose matmuls on the PE (no DMA scatters).
"""

import numpy as np
import ml_dtypes
from contextlib import ExitStack

import concourse.bass as bass
import concourse.tile as tile
from concourse import mybir
from concourse.bass_utils import run_bass_kernel_spmd

F32 = mybir.dt.float32
BF16 = mybir.dt.bfloat16
AF = mybir.ActivationFunctionType
ALU = mybir.AluOpType

P = 128
INP = 2048
HID = 2048
AHID = 1024
NAXIS = 8
NT = 256                 # matmul moving free dim / weight column-block width
KC_H = HID // P          # 16
KC_A = AHID // P         # 8
CT_H = HID // NT         # 8
CT_A = AHID // NT        # 4

MODE = "bf16x2"          # "bf16x2" | "f32"

EPS = 1e-5


# ---------------------------------------------------------------------------
# Workaround: the nix walrus in this container allows only ONE sync wait per
# non-EventSemaphore instruction ("Too many sync wait commands" in
# setupSyncWait). Tile's kernel-tail drain (and occasionally other insts)
# carries several. Split the extras onto single-wait NoOps placed just before
# the instruction on the same engine (per-engine program order preserved).
# ---------------------------------------------------------------------------
def _split_multi_waits(nc):
    n_new = 0
    for f in nc.m.functions:
        for blk in f.blocks:
            out = []
            for inst in blk.instructions:
                si = inst.sync_info
                waits = list(si.on_wait) if si is not None else []
                if len(waits) > 1 and inst.opcode != "EventSemaphore":
                    for w in waits[:-1]:
                        n_new += 1
                        out.append(
                            mybir.InstNoOp(
                                name=f"{inst.name}-wsplit{n_new}",
                                engine=inst.engine,
                                debug=inst.debug,
                                ins=[],
                                outs=[],
                                sync_info=mybir.SyncInfo(on_update=[], on_wait=[w]),
                            )
                        )
                    si.on_wait = [waits[-1]]
                    inst.sync_info = si
                out.append(inst)
            blk.instructions = out
    return n_new


# ---------------------------------------------------------------------------
# Kernel builder
# ---------------------------------------------------------------------------
def _build(mode):
    nc = bass.Bass()
    wdt = BF16 if mode == "bf16x2" else F32
    M = 2 if mode == "bf16x2" else 1  # lhsT free dim (hi,lo) or plain

    def din(name, shape, dt=F32):
        return nc.dram_tensor(name, shape, dt, kind="ExternalInput")

    def dout(name, shape, dt=F32):
        return nc.dram_tensor(name, shape, dt, kind="ExternalOutput")

    vshape = [P, KC_H, 2] if mode == "bf16x2" else [P, KC_H]
    vshape_a = [P, KC_A, 2] if mode == "bf16x2" else [P, KC_A]
    vdt = BF16 if mode == "bf16x2" else F32
    xv_t = din("xv", vshape, vdt)        # in_ chunked [p, kc(,hi/lo)]
    hv_t = din("hv", vshape, vdt)        # h0[a] chunked
    dv_t = din("dv", vshape_a, vdt)      # hdb0 chunked

    c0_t = din("c0r", [1, HID])
    gb_t = din("gbc", [P, 4, KC_H])      # g0,b0,g1,b1 chunked [p, i, kc]
    bdb_t = din("bdbr", [1, 3, AHID])    # bdbx[:,a] + bdbh[:,a]
    bilc_t = din("bilcr", [1, AHID])
    bilh_t = din("bilhr", [1, AHID])

    if mode == "bf16x2":
        Wx_h = din("Wx_hi", [4, INP, HID], BF16)
        Wx_l = din("Wx_lo", [4, INP, HID], BF16)
        Wh_h = din("Wh_hi", [4, HID, HID], BF16)
        Wh_l = din("Wh_lo", [4, HID, HID], BF16)
        Wdbx_h = din("Wdbx_hi", [3, HID, AHID], BF16)
        Wdbx_l = din("Wdbx_lo", [3, HID, AHID], BF16)
        Wdbh_h = din("Wdbh_hi", [3, AHID, AHID], BF16)
        Wdbh_l = din("Wdbh_lo", [3, AHID, AHID], BF16)
        Wict_h = din("Wict_hi", [HID, AHID], BF16)
        Wict_l = din("Wict_lo", [HID, AHID], BF16)
        Wilc_h = din("Wilc_hi", [AHID, AHID], BF16)
        Wilc_l = din("Wilc_lo", [AHID, AHID], BF16)
        Wilh_h = din("Wilh_hi", [AHID, AHID], BF16)
        Wilh_l = din("Wilh_lo", [AHID, AHID], BF16)
    else:
        Wx_h = din("Wx", [4, INP, HID]); Wx_l = None
        Wh_h = din("Wh", [4, HID, HID]); Wh_l = None
        Wdbx_h = din("Wdbx", [3, HID, AHID]); Wdbx_l = None
        Wdbh_h = din("Wdbh", [3, AHID, AHID]); Wdbh_l = None
        Wict_h = din("Wict", [HID, AHID]); Wict_l = None
        Wilc_h = din("Wilc", [AHID, AHID]); Wilc_l = None
        Wilh_h = din("Wilh", [AHID, AHID]); Wilh_l = None

    hm_t = dout("hm", [1, HID])
    cm_t = dout("cm", [1, HID])
    t1_t = dout("t1", [1, AHID])
    t2_t = dout("t2", [1, AHID])

    with tile.TileContext(nc) as tc:
        with ExitStack() as ctx:
            wpool = ctx.enter_context(
                tc.tile_pool(name="wstream", bufs=(6 if mode == "bf16x2" else 3))
            )
            rows = ctx.enter_context(tc.tile_pool(name="rows", bufs=1))
            stmps = ctx.enter_context(tc.tile_pool(name="stmps", bufs=4))
            kcp = ctx.enter_context(tc.tile_pool(name="kcp", bufs=1))
            sm = ctx.enter_context(tc.tile_pool(name="smalls", bufs=1))
            pmm = ctx.enter_context(tc.tile_pool(name="pmm", bufs=4, space="PSUM"))
            ptr = ctx.enter_context(tc.tile_pool(name="ptr", bufs=2, space="PSUM"))
            pcol = ctx.enter_context(tc.tile_pool(name="pcol", bufs=2, space="PSUM"))

            # ---- small input loads -----------------------------------
            xv = kcp.tile(vshape, vdt, tag="xv")
            hv = kcp.tile(vshape, vdt, tag="hv")
            dv = kcp.tile(vshape_a, vdt, tag="dv")
            nc.sync.dma_start(xv[:], xv_t[:])
            nc.sync.dma_start(hv[:], hv_t[:])
            nc.sync.dma_start(dv[:], dv_t[:])

            c0r = rows.tile([1, HID], F32, tag="c0r")
            nc.sync.dma_start(c0r[:], c0_t[:])
            gbc = kcp.tile([P, 4, KC_H], F32, tag="gbc")
            nc.sync.dma_start(gbc[:], gb_t[:])
            bdbr = rows.tile([1, 3, AHID], F32, tag="bdbr")
            nc.sync.dma_start(bdbr[:], bdb_t[:])
            bilcr = rows.tile([1, AHID], F32, tag="bilcr")
            nc.sync.dma_start(bilcr[:], bilc_t[:])
            bilhr = rows.tile([1, AHID], F32, tag="bilhr")
            nc.sync.dma_start(bilhr[:], bilh_t[:])

            ones = sm.tile([1, 1], F32, tag="ones")
            nc.vector.memset(ones[:], 1.0)
            ones2 = sm.tile([2, 1], F32, tag="ones2")
            nc.vector.memset(ones2[:], 1.0)

            # ---- helpers ---------------------------------------------
            def mm_col_block(ps, vec, w_hi_ap, w_lo_ap, kc_n, first, last):
                """Accumulate vec.T @ W[:, ct_block] into psum ps [M, NT].

                w_*_ap: DRAM AP [K, NT] (column block already sliced).
                """
                n_halves = 2 if mode == "bf16x2" else 1
                i = 0
                n_tot = n_halves * kc_n
                for w_ap in (w_hi_ap, w_lo_ap)[:n_halves]:
                    blk = wpool.tile([P, KC_H, NT], wdt, tag="wblk")
                    nc.sync.dma_start(
                        blk[:, :kc_n, :],
                        w_ap.rearrange("(kc p) n -> p kc n", p=P),
                    )
                    for kc in range(kc_n):
                        lhsT = vec[:, kc, :] if mode == "bf16x2" else vec[:, kc : kc + 1]
                        nc.tensor.matmul(
                            ps[:],
                            lhsT,
                            blk[:, kc, :],
                            start=(first and i == 0),
                            stop=(last and i == n_tot - 1),
                        )
                        i += 1

            def collapse(ps):
                """psum [M, NT] -> psum [1, NT] row (hi+lo summed).

                PSUM reads must start at partition 0, and a DVE op may read
                only one PSUM operand, so the two rows are combined on the PE:
                copy [2, NT] to SBUF once, then ones2.T @ t sums the rows.
                """
                if mode != "bf16x2":
                    return ps[0:1, :]
                t = stmps.tile([2, NT], F32, tag="clp")
                nc.vector.tensor_copy(t[:], ps[:])
                pc = pcol.tile([1, NT], F32, tag="pcol")
                nc.tensor.matmul(pc[:], ones2[0:2, 0:1], t[:], start=True, stop=True)
                return pc

            def pe_transpose(row_ap, cpt):
                """[1, cpt*128] f32 row -> psum tile [P, cpt] f32 (v[c*128+p] at [p,c])."""
                pst = ptr.tile([P, KC_H], F32, tag="ptr")
                for c in range(cpt):
                    nc.tensor.matmul(
                        pst[:, c : c + 1],
                        row_ap[0:1, c * P : (c + 1) * P],
                        ones[0:1, 0:1],
                        start=True,
                        stop=True,
                    )
                return pst

            def to_lhsT(src_ap, cpt, name):
                """[P, cpt] f32 (psum or sbuf) -> lhsT tile (f32 or bf16 hi/lo pair)."""
                if mode == "bf16x2":
                    tq = kcp.tile([P, cpt, 2], BF16, tag=name)
                    nc.vector.tensor_copy(tq[:, :, 0], src_ap)
                    hi32 = kcp.tile([P, cpt], F32, tag=name + "32")
                    nc.vector.tensor_copy(hi32[:], tq[:, :, 0])
                    nc.vector.tensor_sub(tq[:, :, 1], src_ap, hi32[:])
                    return tq
                else:
                    tq = kcp.tile([P, cpt], F32, tag=name)
                    nc.vector.tensor_copy(tq[:], src_ap)
                    return tq

            def layernorm_lhsT(src_row, gb_idx, name):
                """LN over [1, HID] src row; gamma/beta applied after the
                row->[P, KC_H] transpose (gbc[:, gb_idx] = gamma chunked,
                gbc[:, gb_idx+1] = beta chunked). Returns lhsT tile."""
                stats = sm.tile([1, CT_H, 6], F32, tag=name + "st")
                for c_ in range(CT_H):
                    nc.vector.bn_stats(
                        stats[:, c_, :], src_row[:, c_ * NT : (c_ + 1) * NT]
                    )
                mv = sm.tile([1, 2], F32, tag=name + "mv")
                nc.vector.bn_aggr(mv[:], stats[:])
                vs = sm.tile([1, 1], F32, tag=name + "vs")
                nc.vector.tensor_scalar_add(vs[:], mv[:, 1:2], EPS)
                sd = sm.tile([1, 1], F32, tag=name + "sd")
                nc.scalar.sqrt(sd[:], vs[:])
                inv = sm.tile([1, 1], F32, tag=name + "inv")
                nc.vector.reciprocal(inv[:], sd[:])
                nmu = sm.tile([1, 1], F32, tag=name + "nmu")
                nc.vector.scalar_tensor_tensor(
                    nmu[:], mv[:, 0:1], -1.0, inv[:], ALU.mult, ALU.mult
                )
                ln0 = rows.tile([1, HID], F32, tag="ln0")
                nc.scalar.activation(
                    ln0[:], src_row[:], AF.Identity, bias=nmu[:], scale=inv[:]
                )
                pst = pe_transpose(ln0, KC_H)
                lnc = kcp.tile([P, KC_H], F32, tag=name + "c")
                nc.vector.tensor_mul(lnc[:], pst[:, :KC_H], gbc[:, gb_idx, :])
                nc.vector.tensor_add(lnc[:], lnc[:], gbc[:, gb_idx + 1, :])
                return to_lhsT(lnc[:], KC_H, name + "kc")

            # ---- stage A: gates --------------------------------------
            ig_r = rows.tile([1, HID], F32, tag="ig")
            gg_r = rows.tile([1, HID], F32, tag="gg")
            og_r = rows.tile([1, HID], F32, tag="og")
            d_r = rows.tile([1, HID], F32, tag="dr")
            rc_r = rows.tile([1, HID], F32, tag="rc")

            gate_rows = {0: ig_r, 2: gg_r, 3: og_r}
            gate_fn = {0: AF.Sigmoid, 2: AF.Tanh, 3: AF.Sigmoid}

            for g in range(4):
                for ct in range(CT_H):
                    cs = slice(ct * NT, (ct + 1) * NT)
                    with nc.named_scope(f"gates_g{g}_ct{ct}"):
                        if g != 1:
                            ps = pmm.tile([M, NT], F32, tag="mmps")
                            mm_col_block(
                                ps, xv,
                                Wx_h[g][:, cs],
                                Wx_l[g][:, cs] if Wx_l is not None else None,
                                KC_H, True, False,
                            )
                            mm_col_block(
                                ps, hv,
                                Wh_h[g][:, cs],
                                Wh_l[g][:, cs] if Wh_l is not None else None,
                                KC_H, False, True,
                            )
                            nc.scalar.activation(
                                gate_rows[g][:, cs], collapse(ps), gate_fn[g]
                            )
                        else:
                            psx = pmm.tile([M, NT], F32, tag="mmps")
                            mm_col_block(
                                psx, xv,
                                Wx_h[1][:, cs],
                                Wx_l[1][:, cs] if Wx_l is not None else None,
                                KC_H, True, True,
                            )
                            psh = pmm.tile([M, NT], F32, tag="mmps")
                            mm_col_block(
                                psh, hv,
                                Wh_h[1][:, cs],
                                Wh_l[1][:, cs] if Wh_l is not None else None,
                                KC_H, True, True,
                            )
                            pxc = collapse(psx)
                            phc = collapse(psh)
                            h1s = stmps.tile([1, NT], F32, tag="h1s")
                            nc.vector.tensor_copy(h1s[:], phc)
                            nc.vector.tensor_sub(d_r[:, cs], pxc, h1s[:])
                            nc.vector.reciprocal(rc_r[:, cs], pxc)

            # ---- stage A chain: c_m, h_m, layernorms ------------------
            with nc.named_scope("gate_chain"):
                # q = d / xg1   (in place of d)
                nc.vector.tensor_mul(d_r[:], d_r[:], rc_r[:])
                # |q|  (in place of rc)
                nc.scalar.activation(rc_r[:], d_r[:], AF.Abs)
                # |q| * c0
                qc_r = rows.tile([1, HID], F32, tag="qc")
                nc.vector.tensor_mul(qc_r[:], rc_r[:], c0r[:])
                # fg_p*c0 = c0 - |q|*c0
                mc_r = rows.tile([1, HID], F32, tag="mc")
                nc.vector.tensor_sub(mc_r[:], c0r[:], qc_r[:])
                # ig * gg  (in place of ig)
                nc.vector.tensor_mul(ig_r[:], ig_r[:], gg_r[:])
                cm_r = rows.tile([1, HID], F32, tag="cmr")
                nc.vector.tensor_add(cm_r[:], mc_r[:], ig_r[:])
                nc.sync.dma_start(cm_t[:], cm_r[:])

            with nc.named_scope("ln_c"):
                lc_kc = layernorm_lhsT(cm_r, 2, "lnc")

            with nc.named_scope("h_m"):
                th_r = rows.tile([1, HID], F32, tag="qc")  # reuse qc slot
                nc.scalar.activation(th_r[:], cm_r[:], AF.Tanh)
                # h_m = og * tanh(c_m)  (in place of og)
                nc.vector.tensor_mul(og_r[:], og_r[:], th_r[:])
                nc.sync.dma_start(hm_t[:], og_r[:])

            with nc.named_scope("ln_h"):
                ht_kc = layernorm_lhsT(og_r, 0, "lnh")

            # ---- stage B: c_t = ln_c @ W_ict --------------------------
            ct_r = rows.tile([1, AHID], F32, tag="ctr")
            for ct in range(CT_A):
                cs = slice(ct * NT, (ct + 1) * NT)
                with nc.named_scope(f"ict_ct{ct}"):
                    ps = pmm.tile([M, NT], F32, tag="mmps")
                    mm_col_block(
                        ps, lc_kc,
                        Wict_h[:][:, cs],
                        Wict_l[:][:, cs] if Wict_l is not None else None,
                        KC_H, True, True,
                    )
                    nc.vector.tensor_copy(ct_r[:, cs], collapse(ps))

            # ---- stage B: pre[j] = h_t @ Wdbx[j] + hdb0 @ Wdbh[j] + b --
            indb_r = rows.tile([1, AHID], F32, tag="indb")
            fndb_r = rows.tile([1, AHID], F32, tag="fndb")
            cndb_r = rows.tile([1, AHID], F32, tag="cndb")
            pre_rows = {0: indb_r, 1: fndb_r, 2: cndb_r}
            pre_fn = {0: AF.Sigmoid, 1: AF.Sigmoid, 2: AF.Tanh}

            for j in range(3):
                for ct in range(CT_A):
                    cs = slice(ct * NT, (ct + 1) * NT)
                    with nc.named_scope(f"db_j{j}_ct{ct}"):
                        ps = pmm.tile([M, NT], F32, tag="mmps")
                        mm_col_block(
                            ps, ht_kc,
                            Wdbx_h[j][:, cs],
                            Wdbx_l[j][:, cs] if Wdbx_l is not None else None,
                            KC_H, True, False,
                        )
                        mm_col_block(
                            ps, dv,
                            Wdbh_h[j][:, cs],
                            Wdbh_l[j][:, cs] if Wdbh_l is not None else None,
                            KC_A, False, True,
                        )
                        srow = stmps.tile([1, NT], F32, tag="stmp2")
                        nc.vector.tensor_add(srow[:], collapse(ps), bdbr[:, j, cs])
                        nc.scalar.activation(pre_rows[j][:, cs], srow[:], pre_fn[j])

            # ---- cells + final matvecs --------------------------------
            with nc.named_scope("cells"):
                icell_r = rows.tile([1, AHID], F32, tag="icell")
                nc.vector.tensor_mul(icell_r[:], indb_r[:], ct_r[:])
                # fn_db * cn_db (in place of fndb)
                nc.vector.tensor_mul(fndb_r[:], fndb_r[:], cndb_r[:])
                # c_cell (in place of cndb)
                nc.vector.tensor_add(cndb_r[:], fndb_r[:], icell_r[:])
                ic_kc = to_lhsT(pe_transpose(icell_r, KC_A)[:, :KC_A], KC_A, "ickc")
                cc_kc = to_lhsT(pe_transpose(cndb_r, KC_A)[:, :KC_A], KC_A, "cckc")

            def il_matvec(vec_kc, w_hi, w_lo, bias_r, out_t, name):
                out_r = rows.tile([1, AHID], F32, tag=name)
                for ct in range(CT_A):
                    cs = slice(ct * NT, (ct + 1) * NT)
                    with nc.named_scope(f"{name}_ct{ct}"):
                        ps = pmm.tile([M, NT], F32, tag="mmps")
                        mm_col_block(
                            ps, vec_kc,
                            w_hi[:][:, cs],
                            w_lo[:][:, cs] if w_lo is not None else None,
                            KC_A, True, True,
                        )
                        nc.vector.tensor_add(
                            out_r[:, cs], collapse(ps), bias_r[:, cs]
                        )
                nc.sync.dma_start(out_t[:], out_r[:])

            il_matvec(ic_kc, Wilc_h, Wilc_l, bilcr, t1_t, "t1")
            il_matvec(cc_kc, Wilh_h, Wilh_l, bilhr, t2_t, "t2")

    _split_multi_waits(nc)
    return nc


_NC_CACHE = {}


def _get_nc(mode):
    if mode not in _NC_CACHE:
        _NC_CACHE[mode] = _build(mode)
    return _NC_CACHE[mode]


# ---------------------------------------------------------------------------
# Host side: shard, run, gather
# ---------------------------------------------------------------------------
def _bf16_split(w):
    """fp32 array -> (hi, lo) bf16 arrays with hi + lo ~= w."""
    hi = w.astype(ml_dtypes.bfloat16)
    lo = (w - hi.astype(np.float32)).astype(ml_dtypes.bfloat16)
    return np.ascontiguousarray(hi), np.ascontiguousarray(lo)


def _chunk_vec(v, kc):
    """[kc*128] -> [128, kc] with elem [p, c] = v[c*128 + p]."""
    return np.ascontiguousarray(v.reshape(kc, P).T)


def _prep_core_inputs(a, mode, in_, h0, c0, hdb0, Wx, Wh, Wdbx, Wdbh, bdb,
                      W_ilc, b_ilc, W_ilh, b_ilh, W_ict, ln_gamma, ln_beta):
    f32 = np.float32
    m = {}
    if mode == "bf16x2":
        def vec2(v, kc):
            hi, lo = _bf16_split(v.astype(f32))
            q = np.empty((P, kc, 2), dtype=ml_dtypes.bfloat16)
            q[:, :, 0] = _chunk_vec(hi, kc)
            q[:, :, 1] = _chunk_vec(lo, kc)
            return q

        m["xv"] = vec2(in_, KC_H)
        m["hv"] = vec2(h0[a], KC_H)
        m["dv"] = vec2(hdb0, KC_A)
        for name, w in (
            ("Wx", Wx[:, a]), ("Wh", Wh[:, a]), ("Wdbx", Wdbx[:, a]),
            ("Wdbh", Wdbh[:, a]), ("Wict", W_ict[a]), ("Wilc", W_ilc[a]),
            ("Wilh", W_ilh[a]),
        ):
            hi, lo = _bf16_split(np.ascontiguousarray(w, dtype=f32))
            m[name + "_hi"] = hi
            m[name + "_lo"] = lo
    else:
        m["xv"] = _chunk_vec(in_.astype(f32), KC_H)
        m["hv"] = _chunk_vec(h0[a].astype(f32), KC_H)
        m["dv"] = _chunk_vec(hdb0.astype(f32), KC_A)
        m["Wx"] = np.ascontiguousarray(Wx[:, a], dtype=f32)
        m["Wh"] = np.ascontiguousarray(Wh[:, a], dtype=f32)
        m["Wdbx"] = np.ascontiguousarray(Wdbx[:, a], dtype=f32)
        m["Wdbh"] = np.ascontiguousarray(Wdbh[:, a], dtype=f32)
        m["Wict"] = np.ascontiguousarray(W_ict[a], dtype=f32)
        m["Wilc"] = np.ascontiguousarray(W_ilc[a], dtype=f32)
        m["Wilh"] = np.ascontiguousarray(W_ilh[a], dtype=f32)

    m["c0r"] = np.ascontiguousarray(c0[a], dtype=f32).reshape(1, HID)
    # gamma/beta chunked: [p, {g0,b0,g1,b1}, kc]
    gb = np.empty((P, 4, KC_H), dtype=f32)
    gb[:, 0] = _chunk_vec(ln_gamma[0, a].astype(f32), KC_H)
    gb[:, 1] = _chunk_vec(ln_beta[0, a].astype(f32), KC_H)
    gb[:, 2] = _chunk_vec(ln_gamma[1, a].astype(f32), KC_H)
    gb[:, 3] = _chunk_vec(ln_beta[1, a].astype(f32), KC_H)
    m["gbc"] = gb
    m["bdbr"] = np.ascontiguousarray(bdb[:, a], dtype=f32).reshape(1, 3, AHID)
    m["bilcr"] = np.ascontiguousarray(b_ilc[a], dtype=f32).reshape(1, AHID)
    m["bilhr"] = np.ascontiguousarray(b_ilh[a], dtype=f32).reshape(1, AHID)
    return m


def _run(inputs, mode=None, **run_kwargs):
    mode = mode or MODE
    f32 = np.float32
    ins = {k: np.asarray(v) for k, v in inputs.items()}
    bdb = ins["bdbx"].astype(f32) + ins["bdbh"].astype(f32)

    in_maps = [
        _prep_core_inputs(
            a, mode, ins["in_"], ins["h0"], ins["c0"], ins["hdb0"],
            ins["Wx"], ins["Wh"], ins["Wdbx"], ins["Wdbh"], bdb,
            ins["W_ilc"], ins["b_ilc"], ins["W_ilh"], ins["b_ilh"],
            ins["W_ict"], ins["ln_gamma"], ins["ln_beta"],
        )
        for a in range(NAXIS)
    ]

    nc = _get_nc(mode)
    res = run_bass_kernel_spmd(nc, in_maps, core_ids=list(range(NAXIS)), **run_kwargs)

    h_m = np.stack([res.results[a]["hm"].reshape(HID) for a in range(NAXIS)])
    c_m = np.stack([res.results[a]["cm"].reshape(HID) for a in range(NAXIS)])
    t1 = np.stack([res.results[a]["t1"].reshape(AHID) for a in range(NAXIS)])
    t2 = np.stack([res.results[a]["t2"].reshape(AHID) for a in range(NAXIS)])

    su_c = t1.astype(np.float64).sum(0)
    su_h = t2.astype(np.float64).sum(0)
    h_db = (1.0 / (1.0 + np.exp(-su_h))) * np.tanh(su_c)

    out = (
        h_m.astype(f32),
        c_m.astype(f32),
        h_db.astype(f32),
    )
    return out, res


def kernel(**inputs):
    out, _ = _run(inputs)
    return out


# revision 8
# speedup vs baseline: 1.1603x; 1.1603x over previous
"""Trainium2 Bass kernel for nn_Branch_Cell (branched LSTM-style cell).

Sharding: expert parallelism over the naxis dimension (naxis == 8 == n_cores).
Core `a` owns axis `a`: it streams that axis's ~189 MB of weights from HBM
(the memory roofline), computes h_m[a], c_m[a] and the partial sums
t1[a], t2[a]; the host sums t1/t2 over axes and applies the final
sigmoid*tanh (8 KB of work, avoids the collective latency floor).

Precision modes:
  'hybrid' (default): weights stream as host-decomposed bf16 hi + bf16 lo
     pairs (same total bytes as fp32; PE runs both halves at full bf16 rate
     with the vector's (hi, lo) pair as a [K, 2] stationary operand) --
     EXCEPT gate 1, whose output feeds a division (1 - |(xg1-hg1)/xg1|)
     that amplifies error near xg1 ~ 0; that gate streams exact fp32.
  'bf16x2': everything bf16 hi/lo.
  'f32': everything fp32 (PE at 4 cyc/row, slightly under HBM rate).

All matvecs keep the vector as the PE stationary operand (weights are the
moving operand), so kernel time is bounded by HBM->SBUF DMA. Weight blocks
are [512 rows x colspan cols] laid out [128p, 4kc, colspan] so every DMA
descriptor moves a 2 KB contiguous run (~95% of DMA line rate).
Elementwise gate/LN math runs on single-partition rows (ACT + DVE);
row->partition-chunk conversion for downstream lhsT operands uses tiny
K=1 transpose matmuls on the PE (no DMA scatters).
"""

import numpy as np
import ml_dtypes
from contextlib import ExitStack

import concourse.bass as bass
import concourse.tile as tile
from concourse import mybir
from concourse.bass_utils import run_bass_kernel_spmd

F32 = mybir.dt.float32
BF16 = mybir.dt.bfloat16
AF = mybir.ActivationFunctionType
ALU = mybir.AluOpType

P = 128
INP = 2048
HID = 2048
AHID = 1024
NAXIS = 8
NT = 512                 # matmul moving free dim
KC_H = HID // P          # 16
KC_A = AHID // P         # 8

MODE = "hybrid"          # "hybrid" | "bf16x2" | "f32"

EPS = 1e-5


# ---------------------------------------------------------------------------
# Workaround: the nix walrus in this container allows only ONE sync wait per
# non-EventSemaphore instruction ("Too many sync wait commands" in
# setupSyncWait). Tile's kernel-tail drain (and occasionally other insts)
# carries several. Split the extras onto single-wait NoOps placed just before
# the instruction on the same engine (per-engine program order preserved).
# ---------------------------------------------------------------------------
def _split_multi_waits(nc):
    n_new = 0
    for f in nc.m.functions:
        for blk in f.blocks:
            out = []
            for inst in blk.instructions:
                si = inst.sync_info
                waits = list(si.on_wait) if si is not None else []
                if len(waits) > 1 and inst.opcode != "EventSemaphore":
                    for w in waits[:-1]:
                        n_new += 1
                        out.append(
                            mybir.InstNoOp(
                                name=f"{inst.name}-wsplit{n_new}",
                                engine=inst.engine,
                                debug=inst.debug,
                                ins=[],
                                outs=[],
                                sync_info=mybir.SyncInfo(on_update=[], on_wait=[w]),
                            )
                        )
                    si.on_wait = [waits[-1]]
                    inst.sync_info = si
                out.append(inst)
            blk.instructions = out
    return n_new


def _gate_prec(mode, g):
    if mode == "f32":
        return "f32"
    if mode == "hybrid" and g == 1:
        return "f32"
    return "bf16"


# ---------------------------------------------------------------------------
# Kernel builder
# ---------------------------------------------------------------------------
def _build(mode):
    nc = bass.Bass()
    prec_b = "f32" if mode == "f32" else "bf16"  # stage-B precision

    def din(name, shape, dt=F32):
        return nc.dram_tensor(name, shape, dt, kind="ExternalInput")

    def dout(name, shape, dt=F32):
        return nc.dram_tensor(name, shape, dt, kind="ExternalOutput")

    def w_pair(name, shape, prec):
        """Declare weight tensor(s): (hi_handle, lo_handle|None)."""
        if prec == "bf16":
            return din(name + "_hi", shape, BF16), din(name + "_lo", shape, BF16)
        return din(name, shape, F32), None

    gate_prec = {g: _gate_prec(mode, g) for g in range(4)}
    Wx_t = {g: w_pair(f"Wx{g}", [INP, HID], gate_prec[g]) for g in range(4)}
    Wh_t = {g: w_pair(f"Wh{g}", [HID, HID], gate_prec[g]) for g in range(4)}
    Wdbx_t = {j: w_pair(f"Wdbx{j}", [HID, AHID], prec_b) for j in range(3)}
    Wdbh_t = {j: w_pair(f"Wdbh{j}", [AHID, AHID], prec_b) for j in range(3)}
    Wict_t = w_pair("Wict", [HID, AHID], prec_b)
    Wilc_t = w_pair("Wilc", [AHID, AHID], prec_b)
    Wilh_t = w_pair("Wilh", [AHID, AHID], prec_b)

    need_bf_vec = any(p == "bf16" for p in gate_prec.values())
    need_f32_vec = any(p == "f32" for p in gate_prec.values())

    xv_t = din("xvq", [P, KC_H, 2], BF16) if need_bf_vec else None
    hv_t = din("hvq", [P, KC_H, 2], BF16) if need_bf_vec else None
    xvf_t = din("xvf", [P, KC_H], F32) if need_f32_vec else None
    hvf_t = din("hvf", [P, KC_H], F32) if need_f32_vec else None
    if prec_b == "bf16":
        dv_t = din("dvq", [P, KC_A, 2], BF16)
    else:
        dv_t = din("dvf", [P, KC_A], F32)

    c0_t = din("c0r", [1, HID])
    gb_t = din("gbc", [P, 4, KC_H])      # g0,b0,g1,b1 chunked [p, i, kc]
    bdb_t = din("bdbr", [1, 3, AHID])    # bdbx[:,a] + bdbh[:,a]
    bilc_t = din("bilcr", [1, AHID])
    bilh_t = din("bilhr", [1, AHID])

    hm_t = dout("hm", [1, HID])
    cm_t = dout("cm", [1, HID])
    t1_t = dout("t1", [1, AHID])
    t2_t = dout("t2", [1, AHID])

    with tile.TileContext(nc) as tc:
        with ExitStack() as ctx:
            wpool = ctx.enter_context(tc.tile_pool(name="wstream", bufs=6))
            rows = ctx.enter_context(tc.tile_pool(name="rows", bufs=1))
            stmps = ctx.enter_context(tc.tile_pool(name="stmps", bufs=3))
            kcp = ctx.enter_context(tc.tile_pool(name="kcp", bufs=1))
            sm = ctx.enter_context(tc.tile_pool(name="smalls", bufs=1))
            pmm = ctx.enter_context(tc.tile_pool(name="pmm", bufs=4, space="PSUM"))
            ptr = ctx.enter_context(tc.tile_pool(name="ptr", bufs=2, space="PSUM"))
            pcol = ctx.enter_context(tc.tile_pool(name="pcol", bufs=2, space="PSUM"))

            # ---- small input loads -----------------------------------
            xv = hv = xvf = hvf = None
            if need_bf_vec:
                xv = kcp.tile([P, KC_H, 2], BF16, tag="xv")
                hv = kcp.tile([P, KC_H, 2], BF16, tag="hv")
                nc.sync.dma_start(xv[:], xv_t[:])
                nc.sync.dma_start(hv[:], hv_t[:])
            if need_f32_vec:
                xvf = kcp.tile([P, KC_H], F32, tag="xvf")
                hvf = kcp.tile([P, KC_H], F32, tag="hvf")
                nc.sync.dma_start(xvf[:], xvf_t[:])
                nc.sync.dma_start(hvf[:], hvf_t[:])
            if prec_b == "bf16":
                dv = kcp.tile([P, KC_A, 2], BF16, tag="dv")
            else:
                dv = kcp.tile([P, KC_A], F32, tag="dv")
            nc.sync.dma_start(dv[:], dv_t[:])

            c0r = rows.tile([1, HID], F32, tag="c0r")
            nc.sync.dma_start(c0r[:], c0_t[:])
            gbc = kcp.tile([P, 4, KC_H], F32, tag="gbc")
            nc.sync.dma_start(gbc[:], gb_t[:])
            bdbr = rows.tile([1, 3, AHID], F32, tag="bdbr")
            nc.sync.dma_start(bdbr[:], bdb_t[:])
            bilcr = rows.tile([1, AHID], F32, tag="bilcr")
            nc.sync.dma_start(bilcr[:], bilc_t[:])
            bilhr = rows.tile([1, AHID], F32, tag="bilhr")
            nc.sync.dma_start(bilhr[:], bilh_t[:])

            ones = sm.tile([1, 1], F32, tag="ones")
            nc.vector.memset(ones[:], 1.0)
            ones2 = sm.tile([2, 1], F32, tag="ones2")
            nc.vector.memset(ones2[:], 1.0)

            # ---- helpers ---------------------------------------------
            def collapse(ps, prec):
                """psum [M, 512] -> psum row [1, 512] AP (hi+lo summed).

                PSUM reads must start at partition 0 and a DVE op may read
                only one PSUM operand, so the two rows are combined on the
                PE: copy [2, NT] to SBUF, then ones2.T @ t sums the rows.
                """
                if prec != "bf16":
                    return ps[0:1, :]
                t = stmps.tile([2, NT], F32, tag="clp")
                nc.vector.tensor_copy(t[:], ps[:])
                pc = pcol.tile([1, NT], F32, tag="pcol")
                nc.tensor.matmul(pc[:], ones2[0:2, 0:1], t[:], start=True, stop=True)
                return pc

            def stream_group(contribs, w_cols, prec, consume, scope):
                """Sum_i vec_i.T @ W_i over full K, streamed in row-contiguous
                blocks; calls consume(col_off, ps, prec) per 512-col output.

                contribs: list of (vec_tile, (w_hi, w_lo), K).
                Weight blocks are [512, colspan] -> [128, 4, colspan] tiles
                (every DMA descriptor run = colspan * dtype = 2 KB).
                """
                colspan = 1024 if prec == "bf16" else 512
                n_ct = colspan // NT
                m_dim = 2 if prec == "bf16" else 1
                n_halves = 2 if prec == "bf16" else 1
                wdt = BF16 if prec == "bf16" else F32
                wtag = "wblk" if prec == "bf16" else "wb32"
                wbufs = 6 if prec == "bf16" else 2
                total_per_ct = sum((K // NT) * 4 * n_halves for _, _, K in contribs)

                for cb in range(w_cols // colspan):
                    with nc.named_scope(f"{scope}_cb{cb}"):
                        pss = [
                            pmm.tile([m_dim, NT], F32, tag="mmps", name="mmps")
                            for _ in range(n_ct)
                        ]
                        cnt = 0
                        for vec, (w_hi, w_lo), K in contribs:
                            for kb in range(K // NT):
                                for w_t in (w_hi, w_lo)[:n_halves]:
                                    blk = wpool.tile(
                                        [P, 4, colspan], wdt, tag=wtag, bufs=wbufs
                                    )
                                    nc.sync.dma_start(
                                        blk[:],
                                        w_t[
                                            kb * NT : (kb + 1) * NT,
                                            cb * colspan : (cb + 1) * colspan,
                                        ].rearrange("(kc p) n -> p kc n", p=P),
                                    )
                                    for kc in range(4):
                                        ki = kb * 4 + kc
                                        lhsT = (
                                            vec[:, ki, :]
                                            if prec == "bf16"
                                            else vec[:, ki : ki + 1]
                                        )
                                        for ct in range(n_ct):
                                            nc.tensor.matmul(
                                                pss[ct][:],
                                                lhsT,
                                                blk[:, kc, ct * NT : (ct + 1) * NT],
                                                start=(cnt // n_ct == 0),
                                                stop=(cnt // n_ct == total_per_ct - 1),
                                            )
                                            cnt += 1
                        for ct in range(n_ct):
                            consume(cb * colspan + ct * NT, pss[ct], prec)

            def pe_transpose(row_ap, cpt):
                """[1, cpt*128] f32 row -> psum tile [P, cpt] f32."""
                pst = ptr.tile([P, KC_H], F32, tag="ptr")
                for c in range(cpt):
                    nc.tensor.matmul(
                        pst[:, c : c + 1],
                        row_ap[0:1, c * P : (c + 1) * P],
                        ones[0:1, 0:1],
                        start=True,
                        stop=True,
                    )
                return pst

            def to_lhsT(src_ap, cpt, name):
                """[P, cpt] f32 (psum/sbuf) -> lhsT tile for stage-B matvecs."""
                if prec_b == "bf16":
                    tq = kcp.tile([P, cpt, 2], BF16, tag=name)
                    nc.vector.tensor_copy(tq[:, :, 0], src_ap)
                    hi32 = kcp.tile([P, cpt], F32, tag=name + "32")
                    nc.vector.tensor_copy(hi32[:], tq[:, :, 0])
                    nc.vector.tensor_sub(tq[:, :, 1], src_ap, hi32[:])
                    return tq
                tq = kcp.tile([P, cpt], F32, tag=name)
                nc.vector.tensor_copy(tq[:], src_ap)
                return tq

            def layernorm_lhsT(src_row, gb_idx, name):
                """LN over [1, HID] row; gamma/beta applied after the
                row->[P, KC_H] transpose. Returns stage-B lhsT tile."""
                stats = sm.tile([1, HID // 512, 6], F32, tag=name + "st")
                for c_ in range(HID // 512):
                    nc.vector.bn_stats(
                        stats[:, c_, :], src_row[:, c_ * 512 : (c_ + 1) * 512]
                    )
                mv = sm.tile([1, 2], F32, tag=name + "mv")
                nc.vector.bn_aggr(mv[:], stats[:])
                vs = sm.tile([1, 1], F32, tag=name + "vs")
                nc.vector.tensor_scalar_add(vs[:], mv[:, 1:2], EPS)
                sd = sm.tile([1, 1], F32, tag=name + "sd")
                nc.scalar.sqrt(sd[:], vs[:])
                inv = sm.tile([1, 1], F32, tag=name + "inv")
                nc.vector.reciprocal(inv[:], sd[:])
                nmu = sm.tile([1, 1], F32, tag=name + "nmu")
                nc.vector.scalar_tensor_tensor(
                    nmu[:], mv[:, 0:1], -1.0, inv[:], ALU.mult, ALU.mult
                )
                ln0 = rows.tile([1, HID], F32, tag="ln0")
                nc.scalar.activation(
                    ln0[:], src_row[:], AF.Identity, bias=nmu[:], scale=inv[:]
                )
                pst = pe_transpose(ln0, KC_H)
                lnc = kcp.tile([P, KC_H], F32, tag=name + "c")
                nc.vector.tensor_mul(lnc[:], pst[:, :KC_H], gbc[:, gb_idx, :])
                nc.vector.tensor_add(lnc[:], lnc[:], gbc[:, gb_idx + 1, :])
                return to_lhsT(lnc[:], KC_H, name + "kc")

            def gvec(prec):
                return (xv, hv) if prec == "bf16" else (xvf, hvf)

            # ---- stage A: gates --------------------------------------
            ig_r = rows.tile([1, HID], F32, tag="ig")
            gg_r = rows.tile([1, HID], F32, tag="gg")
            og_r = rows.tile([1, HID], F32, tag="og")
            d_r = rows.tile([1, HID], F32, tag="dr")
            rc_r = rows.tile([1, HID], F32, tag="rc")

            gate_rows = {0: ig_r, 2: gg_r, 3: og_r}
            gate_fn = {0: AF.Sigmoid, 2: AF.Tanh, 3: AF.Sigmoid}

            for g in range(4):
                pg = gate_prec[g]
                xvec, hvec = gvec(pg)
                if g != 1:
                    def mk_consume(gr=gate_rows[g], fn=gate_fn[g]):
                        def consume(col, ps, prec):
                            nc.scalar.activation(
                                gr[:, col : col + NT], collapse(ps, prec), fn
                            )
                        return consume

                    stream_group(
                        [(xvec, Wx_t[g], INP), (hvec, Wh_t[g], HID)],
                        HID, pg, mk_consume(), f"g{g}",
                    )
                else:
                    # xg1 first: store into d, reciprocal into rc
                    def consume_x(col, ps, prec):
                        pc = collapse(ps, prec)
                        nc.vector.tensor_copy(d_r[:, col : col + NT], pc)
                        nc.vector.reciprocal(rc_r[:, col : col + NT], pc)

                    def consume_h(col, ps, prec):
                        nc.vector.tensor_sub(
                            d_r[:, col : col + NT],
                            d_r[:, col : col + NT],
                            collapse(ps, prec),
                        )

                    stream_group([(xvec, Wx_t[1], INP)], HID, pg, consume_x, "g1x")
                    stream_group([(hvec, Wh_t[1], HID)], HID, pg, consume_h, "g1h")

            # ---- stage A chain: c_m, h_m, layernorms ------------------
            with nc.named_scope("gate_chain"):
                # q = d / xg1   (in place of d)
                nc.vector.tensor_mul(d_r[:], d_r[:], rc_r[:])
                # |q|  (in place of rc)
                nc.scalar.activation(rc_r[:], d_r[:], AF.Abs)
                # |q| * c0   (into d: q dead)
                nc.vector.tensor_mul(d_r[:], rc_r[:], c0r[:])
                # ig * gg  (in place of ig)
                nc.vector.tensor_mul(ig_r[:], ig_r[:], gg_r[:])
                # fg_p*c0 = c0 - |q|*c0   (into rc: |q| dead)
                nc.vector.tensor_sub(rc_r[:], c0r[:], d_r[:])
                cm_r = rows.tile([1, HID], F32, tag="cmr")
                nc.vector.tensor_add(cm_r[:], rc_r[:], ig_r[:])
                nc.sync.dma_start(cm_t[:], cm_r[:])

            with nc.named_scope("ln_c"):
                lc_kc = layernorm_lhsT(cm_r, 2, "lnc")

            with nc.named_scope("h_m"):
                th_r = rows.tile([1, HID], F32, tag="gg")  # reuse gg slot
                nc.scalar.activation(th_r[:], cm_r[:], AF.Tanh)
                # h_m = og * tanh(c_m)  (in place of og)
                nc.vector.tensor_mul(og_r[:], og_r[:], th_r[:])
                nc.sync.dma_start(hm_t[:], og_r[:])

            with nc.named_scope("ln_h"):
                ht_kc = layernorm_lhsT(og_r, 0, "lnh")

            # ---- stage B ---------------------------------------------
            ct_r = rows.tile([1, AHID], F32, tag="ctr")

            def consume_ct(col, ps, prec):
                nc.vector.tensor_copy(ct_r[:, col : col + NT], collapse(ps, prec))

            stream_group([(lc_kc, Wict_t, HID)], AHID, prec_b, consume_ct, "ict")

            indb_r = rows.tile([1, AHID], F32, tag="indb")
            fndb_r = rows.tile([1, AHID], F32, tag="fndb")
            cndb_r = rows.tile([1, AHID], F32, tag="cndb")
            pre_rows = {0: indb_r, 1: fndb_r, 2: cndb_r}
            pre_fn = {0: AF.Sigmoid, 1: AF.Sigmoid, 2: AF.Tanh}

            for j in range(3):
                def mk_consume_pre(jj=j):
                    def consume(col, ps, prec):
                        srow = stmps.tile([1, NT], F32, tag="stmp2")
                        nc.vector.tensor_add(
                            srow[:], collapse(ps, prec),
                            bdbr[:, jj, col : col + NT],
                        )
                        nc.scalar.activation(
                            pre_rows[jj][:, col : col + NT], srow[:], pre_fn[jj]
                        )
                    return consume

                stream_group(
                    [(ht_kc, Wdbx_t[j], HID), (dv, Wdbh_t[j], AHID)],
                    AHID, prec_b, mk_consume_pre(), f"db{j}",
                )

            # ---- cells + final matvecs --------------------------------
            with nc.named_scope("cells"):
                # i_cell = in_db * c_t (in place of indb)
                nc.vector.tensor_mul(indb_r[:], indb_r[:], ct_r[:])
                # fn_db * cn_db (in place of fndb)
                nc.vector.tensor_mul(fndb_r[:], fndb_r[:], cndb_r[:])
                # c_cell (in place of cndb)
                nc.vector.tensor_add(cndb_r[:], fndb_r[:], indb_r[:])
                ic_kc = to_lhsT(pe_transpose(indb_r, KC_A)[:, :KC_A], KC_A, "ickc")
                cc_kc = to_lhsT(pe_transpose(cndb_r, KC_A)[:, :KC_A], KC_A, "cckc")

            def mk_consume_il(bias_r, out_t):
                def consume(col, ps, prec):
                    srow = stmps.tile([1, NT], F32, tag="stmp3")
                    nc.vector.tensor_add(
                        srow[:], collapse(ps, prec), bias_r[:, col : col + NT]
                    )
                    nc.sync.dma_start(out_t[:, col : col + NT], srow[:])
                return consume

            stream_group(
                [(ic_kc, Wilc_t, AHID)], AHID, prec_b,
                mk_consume_il(bilcr, t1_t), "t1",
            )
            stream_group(
                [(cc_kc, Wilh_t, AHID)], AHID, prec_b,
                mk_consume_il(bilhr, t2_t), "t2",
            )

    _split_multi_waits(nc)
    return nc


_NC_CACHE = {}


def _get_nc(mode):
    if mode not in _NC_CACHE:
        _NC_CACHE[mode] = _build(mode)
    return _NC_CACHE[mode]


# ---------------------------------------------------------------------------
# Host side: shard, run, gather
# ---------------------------------------------------------------------------
def _bf16_split(w):
    """fp32 array -> (hi, lo) bf16 arrays with hi + lo ~= w (round-to-nearest).

    Bit-twiddled for speed: hi = RNE-round to bf16; lo = RNE(w - hi).
    """
    w = np.ascontiguousarray(w, dtype=np.float32)
    u = w.view(np.uint32)
    rhi = (u + 0x7FFF + ((u >> 16) & 1)) & 0xFFFF0000
    hi32 = rhi.view(np.float32)
    hi = (rhi >> 16).astype(np.uint16).view(ml_dtypes.bfloat16)
    lo = (w - hi32).astype(ml_dtypes.bfloat16)
    return np.ascontiguousarray(hi), np.ascontiguousarray(lo)


def _chunk_vec(v, kc):
    """[kc*128] -> [128, kc] with elem [p, c] = v[c*128 + p]."""
    return np.ascontiguousarray(np.asarray(v, np.float32).reshape(kc, P).T)


def _vec_pair(v, kc):
    hi, lo = _bf16_split(np.asarray(v, np.float32).reshape(-1))
    q = np.empty((P, kc, 2), dtype=ml_dtypes.bfloat16)
    q[:, :, 0] = np.asarray(hi).reshape(kc, P).T
    q[:, :, 1] = np.asarray(lo).reshape(kc, P).T
    return q


def _prep_core_inputs(a, mode, ins, bdb):
    f32 = np.float32
    m = {}
    gate_prec = {g: _gate_prec(mode, g) for g in range(4)}
    prec_b = "f32" if mode == "f32" else "bf16"

    def put_w(name, w, prec):
        w = np.ascontiguousarray(w, dtype=f32)
        if prec == "bf16":
            hi, lo = _bf16_split(w)
            m[name + "_hi"] = hi
            m[name + "_lo"] = lo
        else:
            m[name] = w

    for g in range(4):
        put_w(f"Wx{g}", ins["Wx"][g, a], gate_prec[g])
        put_w(f"Wh{g}", ins["Wh"][g, a], gate_prec[g])
    for j in range(3):
        put_w(f"Wdbx{j}", ins["Wdbx"][j, a], prec_b)
        put_w(f"Wdbh{j}", ins["Wdbh"][j, a], prec_b)
    put_w("Wict", ins["W_ict"][a], prec_b)
    put_w("Wilc", ins["W_ilc"][a], prec_b)
    put_w("Wilh", ins["W_ilh"][a], prec_b)

    need_bf_vec = any(p == "bf16" for p in gate_prec.values())
    need_f32_vec = any(p == "f32" for p in gate_prec.values())
    if need_bf_vec:
        m["xvq"] = _vec_pair(ins["in_"], KC_H)
        m["hvq"] = _vec_pair(ins["h0"][a], KC_H)
    if need_f32_vec:
        m["xvf"] = _chunk_vec(ins["in_"], KC_H)
        m["hvf"] = _chunk_vec(ins["h0"][a], KC_H)
    if prec_b == "bf16":
        m["dvq"] = _vec_pair(ins["hdb0"], KC_A)
    else:
        m["dvf"] = _chunk_vec(ins["hdb0"], KC_A)

    m["c0r"] = np.ascontiguousarray(ins["c0"][a], dtype=f32).reshape(1, HID)
    gb = np.empty((P, 4, KC_H), dtype=f32)
    gb[:, 0] = _chunk_vec(ins["ln_gamma"][0, a], KC_H)
    gb[:, 1] = _chunk_vec(ins["ln_beta"][0, a], KC_H)
    gb[:, 2] = _chunk_vec(ins["ln_gamma"][1, a], KC_H)
    gb[:, 3] = _chunk_vec(ins["ln_beta"][1, a], KC_H)
    m["gbc"] = gb
    m["bdbr"] = np.ascontiguousarray(bdb[:, a], dtype=f32).reshape(1, 3, AHID)
    m["bilcr"] = np.ascontiguousarray(ins["b_ilc"][a], dtype=f32).reshape(1, AHID)
    m["bilhr"] = np.ascontiguousarray(ins["b_ilh"][a], dtype=f32).reshape(1, AHID)
    return m


def _run(inputs, mode=None, **run_kwargs):
    mode = mode or MODE
    f32 = np.float32
    ins = {k: np.asarray(v) for k, v in inputs.items()}
    bdb = ins["bdbx"].astype(f32) + ins["bdbh"].astype(f32)

    in_maps = [_prep_core_inputs(a, mode, ins, bdb) for a in range(NAXIS)]

    nc = _get_nc(mode)
    res = run_bass_kernel_spmd(nc, in_maps, core_ids=list(range(NAXIS)), **run_kwargs)

    h_m = np.stack([res.results[a]["hm"].reshape(HID) for a in range(NAXIS)])
    c_m = np.stack([res.results[a]["cm"].reshape(HID) for a in range(NAXIS)])
    t1 = np.stack([res.results[a]["t1"].reshape(AHID) for a in range(NAXIS)])
    t2 = np.stack([res.results[a]["t2"].reshape(AHID) for a in range(NAXIS)])

    su_c = t1.astype(np.float64).sum(0)
    su_h = t2.astype(np.float64).sum(0)
    h_db = (1.0 / (1.0 + np.exp(-su_h))) * np.tanh(su_c)

    return (h_m.astype(f32), c_m.astype(f32), h_db.astype(f32)), res


def kernel(**inputs):
    out, _ = _run(inputs)
    return out


# revision 11
# speedup vs baseline: 1.2192x; 1.0507x over previous
"""Trainium2 Bass kernel for nn_Branch_Cell (branched LSTM-style cell).

Sharding: expert parallelism over the naxis dimension (naxis == 8 == n_cores).
Core `a` owns axis `a`: it streams that axis's ~189 MB of weights from HBM
(the memory roofline), computes h_m[a], c_m[a] and the partial sums
t1[a], t2[a]; the host sums t1/t2 over axes and applies the final
sigmoid*tanh (8 KB of work, avoids the collective latency floor).

Precision modes:
  'hybrid' (default): weights stream as host-decomposed bf16 hi + bf16 lo
     pairs (same total bytes as fp32; PE runs both halves at full bf16 rate
     with the vector's (hi, lo) pair as a [K, 2] stationary operand) --
     EXCEPT gate 1, whose output feeds a division (1 - |(xg1-hg1)/xg1|)
     that amplifies error near xg1 ~ 0; that gate streams exact fp32.
  'bf16x2': everything bf16 hi/lo.
  'f32': everything fp32 (PE at 4 cyc/row, slightly under HBM rate).

All matvecs keep the vector as the PE stationary operand (weights are the
moving operand), so kernel time is bounded by HBM->SBUF DMA. Weight blocks
are [512 rows x colspan cols] laid out [128p, 4kc, colspan] so every DMA
descriptor moves a 2 KB contiguous run (~95% of DMA line rate).
Elementwise gate/LN math runs on single-partition rows (ACT + DVE);
row->partition-chunk conversion for downstream lhsT operands uses tiny
K=1 transpose matmuls on the PE (no DMA scatters).
"""

import numpy as np
import ml_dtypes
from contextlib import ExitStack

import concourse.bass as bass
import concourse.tile as tile
from concourse import mybir
from concourse.bass_utils import run_bass_kernel_spmd

F32 = mybir.dt.float32
F32R = mybir.dt.float32r
BF16 = mybir.dt.bfloat16
AF = mybir.ActivationFunctionType
ALU = mybir.AluOpType

P = 128
INP = 2048
HID = 2048
AHID = 1024
NAXIS = 8
NT = 512                 # matmul moving free dim
KC_H = HID // P          # 16
KC_A = AHID // P         # 8

MODE = "hybrid"          # "hybrid" | "hybrid_r" | "f32r" | "bf16x2" | "f32"

# Weight dtypes per precision name
_WDT = None  # set below after mybir import resolution

import os as _os
if _os.environ.get("LDWOPT", "0") == "1":
    # walrus's own default is --enable-ldw-opt=true; bass pins it false.
    # Allow re-enabling for A/B (validated against the f64 oracle).
    import concourse.bass_utils as _bu
    _orig_run_command = _bu.run_command
    def _run_command_ldwopt(argv, **kw):
        argv = ["--enable-ldw-opt=true" if a == "--enable-ldw-opt=false" else a
                for a in argv]
        return _orig_run_command(argv, **kw)
    _bu.run_command = _run_command_ldwopt

EPS = 1e-5


# ---------------------------------------------------------------------------
# Workaround: the nix walrus in this container allows only ONE sync wait per
# non-EventSemaphore instruction ("Too many sync wait commands" in
# setupSyncWait). Tile's kernel-tail drain (and occasionally other insts)
# carries several. Split the extras onto single-wait NoOps placed just before
# the instruction on the same engine (per-engine program order preserved).
# ---------------------------------------------------------------------------
def _split_multi_waits(nc):
    n_new = 0
    for f in nc.m.functions:
        for blk in f.blocks:
            out = []
            for inst in blk.instructions:
                si = inst.sync_info
                waits = list(si.on_wait) if si is not None else []
                if len(waits) > 1 and inst.opcode != "EventSemaphore":
                    for w in waits[:-1]:
                        n_new += 1
                        out.append(
                            mybir.InstNoOp(
                                name=f"{inst.name}-wsplit{n_new}",
                                engine=inst.engine,
                                debug=inst.debug,
                                ins=[],
                                outs=[],
                                sync_info=mybir.SyncInfo(on_update=[], on_wait=[w]),
                            )
                        )
                    si.on_wait = [waits[-1]]
                    inst.sync_info = si
                out.append(inst)
            blk.instructions = out
    return n_new


def _gate_prec(mode, g):
    if mode == "f32":
        return "f32"
    if mode == "f32r":
        return "f32r"
    if mode in ("hybrid", "hybrid_r") and g == 1:
        return "f32"
    return "f32r" if mode == "hybrid_r" else "bf16"


def _prec_b(mode):
    if mode == "f32":
        return "f32"
    if mode in ("f32r", "hybrid_r"):
        return "f32r"
    return "bf16"


# ---------------------------------------------------------------------------
# Kernel builder
# ---------------------------------------------------------------------------
def _build(mode):
    nc = bass.Bass()
    prec_b = _prec_b(mode)

    def din(name, shape, dt=F32):
        return nc.dram_tensor(name, shape, dt, kind="ExternalInput")

    def dout(name, shape, dt=F32):
        return nc.dram_tensor(name, shape, dt, kind="ExternalOutput")

    def w_pair(name, shape, prec):
        """Declare weight tensor(s): (hi_handle, lo_handle|None)."""
        if prec == "bf16":
            return din(name + "_hi", shape, BF16), din(name + "_lo", shape, BF16)
        return din(name, shape, F32 if prec == "f32" else F32R), None

    gate_prec = {g: _gate_prec(mode, g) for g in range(4)}
    Wx_t = {g: w_pair(f"Wx{g}", [INP, HID], gate_prec[g]) for g in range(4)}
    Wh_t = {g: w_pair(f"Wh{g}", [HID, HID], gate_prec[g]) for g in range(4)}
    Wdbx_t = {j: w_pair(f"Wdbx{j}", [HID, AHID], prec_b) for j in range(3)}
    Wdbh_t = {j: w_pair(f"Wdbh{j}", [AHID, AHID], prec_b) for j in range(3)}
    Wict_t = w_pair("Wict", [HID, AHID], prec_b)
    Wilc_t = w_pair("Wilc", [AHID, AHID], prec_b)
    Wilh_t = w_pair("Wilh", [AHID, AHID], prec_b)

    need_bf_vec = any(p == "bf16" for p in gate_prec.values())
    need_f32_vec = any(p == "f32" for p in gate_prec.values())
    need_f32r_vec = any(p == "f32r" for p in gate_prec.values())

    xv_t = din("xvq", [P, KC_H, 2], BF16) if need_bf_vec else None
    hv_t = din("hvq", [P, KC_H, 2], BF16) if need_bf_vec else None
    xvf_t = din("xvf", [P, KC_H], F32) if need_f32_vec else None
    hvf_t = din("hvf", [P, KC_H], F32) if need_f32_vec else None
    xvr_t = din("xvr", [P, KC_H], F32R) if need_f32r_vec else None
    hvr_t = din("hvr", [P, KC_H], F32R) if need_f32r_vec else None
    if prec_b == "bf16":
        dv_t = din("dvq", [P, KC_A, 2], BF16)
    elif prec_b == "f32r":
        dv_t = din("dvr", [P, KC_A], F32R)
    else:
        dv_t = din("dvf", [P, KC_A], F32)

    c0_t = din("c0r", [1, HID])
    gb_t = din("gbc", [P, 4, KC_H])      # g0,b0,g1,b1 chunked [p, i, kc]
    bdb_t = din("bdbr", [1, 3, AHID])    # bdbx[:,a] + bdbh[:,a]
    bilc_t = din("bilcr", [1, AHID])
    bilh_t = din("bilhr", [1, AHID])

    hm_t = dout("hm", [1, HID])
    cm_t = dout("cm", [1, HID])
    t1_t = dout("t1", [1, AHID])
    t2_t = dout("t2", [1, AHID])

    with tile.TileContext(nc) as tc:
        with ExitStack() as ctx:
            wpool = ctx.enter_context(tc.tile_pool(name="wstream", bufs=6))
            rows = ctx.enter_context(tc.tile_pool(name="rows", bufs=1))
            stmps = ctx.enter_context(tc.tile_pool(name="stmps", bufs=3))
            kcp = ctx.enter_context(tc.tile_pool(name="kcp", bufs=1))
            sm = ctx.enter_context(tc.tile_pool(name="smalls", bufs=1))
            pmm = ctx.enter_context(tc.tile_pool(name="pmm", bufs=4, space="PSUM"))
            ptr = ctx.enter_context(tc.tile_pool(name="ptr", bufs=2, space="PSUM"))
            pcol = ctx.enter_context(tc.tile_pool(name="pcol", bufs=2, space="PSUM"))

            # ---- small input loads -----------------------------------
            xv = hv = xvf = hvf = xvr = hvr = None
            if need_bf_vec:
                xv = kcp.tile([P, KC_H, 2], BF16, tag="xv")
                hv = kcp.tile([P, KC_H, 2], BF16, tag="hv")
                nc.sync.dma_start(xv[:], xv_t[:])
                nc.sync.dma_start(hv[:], hv_t[:])
            if need_f32_vec:
                xvf = kcp.tile([P, KC_H], F32, tag="xvf")
                hvf = kcp.tile([P, KC_H], F32, tag="hvf")
                nc.sync.dma_start(xvf[:], xvf_t[:])
                nc.sync.dma_start(hvf[:], hvf_t[:])
            if need_f32r_vec:
                xvr = kcp.tile([P, KC_H], F32R, tag="xvr")
                hvr = kcp.tile([P, KC_H], F32R, tag="hvr")
                nc.sync.dma_start(xvr[:], xvr_t[:])
                nc.sync.dma_start(hvr[:], hvr_t[:])
            if prec_b == "bf16":
                dv = kcp.tile([P, KC_A, 2], BF16, tag="dv")
            elif prec_b == "f32r":
                dv = kcp.tile([P, KC_A], F32R, tag="dv")
            else:
                dv = kcp.tile([P, KC_A], F32, tag="dv")
            nc.sync.dma_start(dv[:], dv_t[:])

            c0r = rows.tile([1, HID], F32, tag="c0r")
            nc.sync.dma_start(c0r[:], c0_t[:])
            gbc = kcp.tile([P, 4, KC_H], F32, tag="gbc")
            nc.sync.dma_start(gbc[:], gb_t[:])
            bdbr = rows.tile([1, 3, AHID], F32, tag="bdbr")
            nc.sync.dma_start(bdbr[:], bdb_t[:])
            bilcr = rows.tile([1, AHID], F32, tag="bilcr")
            nc.sync.dma_start(bilcr[:], bilc_t[:])
            bilhr = rows.tile([1, AHID], F32, tag="bilhr")
            nc.sync.dma_start(bilhr[:], bilh_t[:])

            ones = sm.tile([1, 1], F32, tag="ones")
            nc.vector.memset(ones[:], 1.0)
            ones2 = sm.tile([2, 1], F32, tag="ones2")
            nc.vector.memset(ones2[:], 1.0)

            # ---- helpers ---------------------------------------------
            def collapse(ps, prec):
                """psum [M, 512] -> psum row [1, 512] AP (hi+lo summed).

                PSUM reads must start at partition 0 and a DVE op may read
                only one PSUM operand, so the two rows are combined on the
                PE: copy [2, NT] to SBUF, then ones2.T @ t sums the rows.
                """
                if prec != "bf16":
                    return ps[0:1, :]
                t = stmps.tile([2, NT], F32, tag="clp")
                nc.vector.tensor_copy(t[:], ps[:])
                pc = pcol.tile([1, NT], F32, tag="pcol")
                nc.tensor.matmul(pc[:], ones2[0:2, 0:1], t[:], start=True, stop=True)
                return pc

            # Consume-stage software pipeline: each column-block's psum
            # collapse/activation is deferred until the NEXT block's matmuls
            # have been emitted, so the in-order PE never stalls on the DVE
            # copy feeding the collapse matmul.
            deferred = []

            def flush_deferred():
                while deferred:
                    deferred.pop(0)()

            def stream_group(contribs, w_cols, prec, consume, scope):
                """Sum_i vec_i.T @ W_i over full K, streamed in row-contiguous
                blocks; calls consume(col_off, ps, prec) per 512-col output
                (deferred by one block).

                contribs: list of (vec_tile, (w_hi, w_lo), K).
                Weight blocks are [512, colspan] -> [128, 4, colspan] tiles
                (every DMA descriptor run = colspan * dtype = 2 KB).
                """
                colspan = 1024 if prec == "bf16" else 512
                n_ct = colspan // NT
                m_dim = 2 if prec == "bf16" else 1
                n_halves = 2 if prec == "bf16" else 1
                wdt = {"bf16": BF16, "f32": F32, "f32r": F32R}[prec]
                wtag = "wblk"
                wbufs = 8
                total_per_ct = sum((K // NT) * 4 * n_halves for _, _, K in contribs)

                for cb in range(w_cols // colspan):
                    with nc.named_scope(f"{scope}_cb{cb}"):
                        pss = [
                            pmm.tile([m_dim, NT], F32, tag="mmps", name="mmps")
                            for _ in range(n_ct)
                        ]
                        cnt = 0
                        n_blk = 0
                        for vec, (w_hi, w_lo), K in contribs:
                            for kb in range(K // NT):
                                for w_t in (w_hi, w_lo)[:n_halves]:
                                    blk = wpool.tile(
                                        [P, 4, colspan], wdt, tag=wtag, bufs=wbufs
                                    )
                                    nc.sync.dma_start(
                                        blk[:],
                                        w_t[
                                            kb * NT : (kb + 1) * NT,
                                            cb * colspan : (cb + 1) * colspan,
                                        ].rearrange("(kc p) n -> p kc n", p=P),
                                    )
                                    for kc in range(4):
                                        ki = kb * 4 + kc
                                        lhsT = (
                                            vec[:, ki, :]
                                            if prec == "bf16"
                                            else vec[:, ki : ki + 1]
                                        )
                                        for ct in range(n_ct):
                                            nc.tensor.matmul(
                                                pss[ct][:],
                                                lhsT,
                                                blk[:, kc, ct * NT : (ct + 1) * NT],
                                                start=(cnt // n_ct == 0),
                                                stop=(cnt // n_ct == total_per_ct - 1),
                                            )
                                            cnt += 1
                                    n_blk += 1
                                    if n_blk == 1:
                                        flush_deferred()

                        def _consume_cb(cb=cb, pss=pss):
                            for ct in range(n_ct):
                                consume(cb * colspan + ct * NT, pss[ct], prec)

                        deferred.append(_consume_cb)

            def pe_transpose(row_ap, cpt):
                """[1, cpt*128] f32 row -> psum tile [P, cpt] f32."""
                pst = ptr.tile([P, KC_H], F32, tag="ptr")
                for c in range(cpt):
                    nc.tensor.matmul(
                        pst[:, c : c + 1],
                        row_ap[0:1, c * P : (c + 1) * P],
                        ones[0:1, 0:1],
                        start=True,
                        stop=True,
                    )
                return pst

            def to_lhsT(src_ap, cpt, name):
                """[P, cpt] f32 (psum/sbuf) -> lhsT tile for stage-B matvecs."""
                if prec_b == "bf16":
                    tq = kcp.tile([P, cpt, 2], BF16, tag=name)
                    nc.vector.tensor_copy(tq[:, :, 0], src_ap)
                    hi32 = kcp.tile([P, cpt], F32, tag=name + "32")
                    nc.vector.tensor_copy(hi32[:], tq[:, :, 0])
                    nc.vector.tensor_sub(tq[:, :, 1], src_ap, hi32[:])
                    return tq
                wdt = F32R if prec_b == "f32r" else F32
                tq = kcp.tile([P, cpt], wdt, tag=name)
                nc.vector.tensor_copy(tq[:], src_ap)
                return tq

            def layernorm_lhsT(src_row, gb_idx, name):
                """LN over [1, HID] row; gamma/beta applied after the
                row->[P, KC_H] transpose. Returns stage-B lhsT tile."""
                stats = sm.tile([1, HID // 512, 6], F32, tag=name + "st")
                for c_ in range(HID // 512):
                    nc.vector.bn_stats(
                        stats[:, c_, :], src_row[:, c_ * 512 : (c_ + 1) * 512]
                    )
                mv = sm.tile([1, 2], F32, tag=name + "mv")
                nc.vector.bn_aggr(mv[:], stats[:])
                vs = sm.tile([1, 1], F32, tag=name + "vs")
                nc.vector.tensor_scalar_add(vs[:], mv[:, 1:2], EPS)
                sd = sm.tile([1, 1], F32, tag=name + "sd")
                nc.scalar.sqrt(sd[:], vs[:])
                inv = sm.tile([1, 1], F32, tag=name + "inv")
                nc.vector.reciprocal(inv[:], sd[:])
                nmu = sm.tile([1, 1], F32, tag=name + "nmu")
                nc.vector.scalar_tensor_tensor(
                    nmu[:], mv[:, 0:1], -1.0, inv[:], ALU.mult, ALU.mult
                )
                ln0 = rows.tile([1, HID], F32, tag="ln0")
                nc.scalar.activation(
                    ln0[:], src_row[:], AF.Identity, bias=nmu[:], scale=inv[:]
                )
                pst = pe_transpose(ln0, KC_H)
                lnc = kcp.tile([P, KC_H], F32, tag=name + "c")
                nc.vector.tensor_mul(lnc[:], pst[:, :KC_H], gbc[:, gb_idx, :])
                nc.vector.tensor_add(lnc[:], lnc[:], gbc[:, gb_idx + 1, :])
                return to_lhsT(lnc[:], KC_H, name + "kc")

            def gvec(prec):
                if prec == "bf16":
                    return (xv, hv)
                if prec == "f32r":
                    return (xvr, hvr)
                return (xvf, hvf)

            # ---- stage A: gates --------------------------------------
            ig_r = rows.tile([1, HID], F32, tag="ig")
            gg_r = rows.tile([1, HID], F32, tag="gg")
            og_r = rows.tile([1, HID], F32, tag="og")
            d_r = rows.tile([1, HID], F32, tag="dr")
            rc_r = rows.tile([1, HID], F32, tag="rc")

            gate_rows = {0: ig_r, 2: gg_r, 3: og_r}
            gate_fn = {0: AF.Sigmoid, 2: AF.Tanh, 3: AF.Sigmoid}

            def emit_gate(g):
                pg = gate_prec[g]
                xvec, hvec = gvec(pg)

                def mk_consume(gr=gate_rows[g], fn=gate_fn[g]):
                    def consume(col, ps, prec):
                        nc.scalar.activation(
                            gr[:, col : col + NT], collapse(ps, prec), fn
                        )
                    return consume

                stream_group(
                    [(xvec, Wx_t[g], INP), (hvec, Wh_t[g], HID)],
                    HID, pg, mk_consume(), f"g{g}",
                )

            def consume_x(col, ps, prec):
                pc = collapse(ps, prec)
                nc.vector.tensor_copy(d_r[:, col : col + NT], pc)
                nc.vector.reciprocal(rc_r[:, col : col + NT], pc)

            def consume_h(col, ps, prec):
                nc.vector.tensor_sub(
                    d_r[:, col : col + NT],
                    d_r[:, col : col + NT],
                    collapse(ps, prec),
                )

            pg1 = gate_prec[1]
            xv1, hv1 = gvec(pg1)
            # interleave the (PE-heavier) fp32 gate-1 passes between the
            # bf16 gates so the PE hump is absorbed by the DMA prefetch bufs
            emit_gate(0)
            stream_group([(xv1, Wx_t[1], INP)], HID, pg1, consume_x, "g1x")
            emit_gate(2)
            stream_group([(hv1, Wh_t[1], HID)], HID, pg1, consume_h, "g1h")
            emit_gate(3)

            # ---- stage A chain: c_m, h_m, layernorms ------------------
            flush_deferred()
            with nc.named_scope("gate_chain"):
                # q = d / xg1   (in place of d)
                nc.vector.tensor_mul(d_r[:], d_r[:], rc_r[:])
                # |q|  (in place of rc)
                nc.scalar.activation(rc_r[:], d_r[:], AF.Abs)
                # |q| * c0   (into d: q dead)
                nc.vector.tensor_mul(d_r[:], rc_r[:], c0r[:])
                # ig * gg  (in place of ig)
                nc.vector.tensor_mul(ig_r[:], ig_r[:], gg_r[:])
                # fg_p*c0 = c0 - |q|*c0   (into rc: |q| dead)
                nc.vector.tensor_sub(rc_r[:], c0r[:], d_r[:])
                cm_r = rows.tile([1, HID], F32, tag="cmr")
                nc.vector.tensor_add(cm_r[:], rc_r[:], ig_r[:])
                nc.sync.dma_start(cm_t[:], cm_r[:])

            with nc.named_scope("ln_c"):
                lc_kc = layernorm_lhsT(cm_r, 2, "lnc")

            with nc.named_scope("h_m"):
                th_r = rows.tile([1, HID], F32, tag="gg")  # reuse gg slot
                nc.scalar.activation(th_r[:], cm_r[:], AF.Tanh)
                # h_m = og * tanh(c_m)  (in place of og)
                nc.vector.tensor_mul(og_r[:], og_r[:], th_r[:])
                nc.sync.dma_start(hm_t[:], og_r[:])

            with nc.named_scope("ln_h"):
                ht_kc = layernorm_lhsT(og_r, 0, "lnh")

            # ---- stage B ---------------------------------------------
            ct_r = rows.tile([1, AHID], F32, tag="ctr")

            def consume_ct(col, ps, prec):
                nc.vector.tensor_copy(ct_r[:, col : col + NT], collapse(ps, prec))

            stream_group([(lc_kc, Wict_t, HID)], AHID, prec_b, consume_ct, "ict")

            indb_r = rows.tile([1, AHID], F32, tag="indb")
            fndb_r = rows.tile([1, AHID], F32, tag="fndb")
            cndb_r = rows.tile([1, AHID], F32, tag="cndb")
            pre_rows = {0: indb_r, 1: fndb_r, 2: cndb_r}
            pre_fn = {0: AF.Sigmoid, 1: AF.Sigmoid, 2: AF.Tanh}

            for j in range(3):
                def mk_consume_pre(jj=j):
                    def consume(col, ps, prec):
                        srow = stmps.tile([1, NT], F32, tag="stmp2")
                        nc.vector.tensor_add(
                            srow[:], collapse(ps, prec),
                            bdbr[:, jj, col : col + NT],
                        )
                        nc.scalar.activation(
                            pre_rows[jj][:, col : col + NT], srow[:], pre_fn[jj]
                        )
                    return consume

                stream_group(
                    [(ht_kc, Wdbx_t[j], HID), (dv, Wdbh_t[j], AHID)],
                    AHID, prec_b, mk_consume_pre(), f"db{j}",
                )

            # ---- cells + final matvecs --------------------------------
            flush_deferred()
            with nc.named_scope("cells"):
                # i_cell = in_db * c_t (in place of indb)
                nc.vector.tensor_mul(indb_r[:], indb_r[:], ct_r[:])
                # fn_db * cn_db (in place of fndb)
                nc.vector.tensor_mul(fndb_r[:], fndb_r[:], cndb_r[:])
                # c_cell (in place of cndb)
                nc.vector.tensor_add(cndb_r[:], fndb_r[:], indb_r[:])
                ic_kc = to_lhsT(pe_transpose(indb_r, KC_A)[:, :KC_A], KC_A, "ickc")
                cc_kc = to_lhsT(pe_transpose(cndb_r, KC_A)[:, :KC_A], KC_A, "cckc")

            def mk_consume_il(bias_r, out_t):
                def consume(col, ps, prec):
                    srow = stmps.tile([1, NT], F32, tag="stmp3")
                    nc.vector.tensor_add(
                        srow[:], collapse(ps, prec), bias_r[:, col : col + NT]
                    )
                    nc.sync.dma_start(out_t[:, col : col + NT], srow[:])
                return consume

            stream_group(
                [(ic_kc, Wilc_t, AHID)], AHID, prec_b,
                mk_consume_il(bilcr, t1_t), "t1",
            )
            stream_group(
                [(cc_kc, Wilh_t, AHID)], AHID, prec_b,
                mk_consume_il(bilhr, t2_t), "t2",
            )
            flush_deferred()

    _split_multi_waits(nc)
    return nc


_NC_CACHE = {}


def _get_nc(mode):
    if mode not in _NC_CACHE:
        _NC_CACHE[mode] = _build(mode)
    return _NC_CACHE[mode]


# ---------------------------------------------------------------------------
# Host side: shard, run, gather
# ---------------------------------------------------------------------------
def _bf16_split(w):
    """fp32 array -> (hi, lo) bf16 arrays with hi + lo ~= w (round-to-nearest).

    Bit-twiddled for speed: hi = RNE-round to bf16; lo = RNE(w - hi).
    """
    w = np.ascontiguousarray(w, dtype=np.float32)
    u = w.view(np.uint32)
    rhi = (u + 0x7FFF + ((u >> 16) & 1)) & 0xFFFF0000
    hi32 = rhi.view(np.float32)
    hi = (rhi >> 16).astype(np.uint16).view(ml_dtypes.bfloat16)
    lo = (w - hi32).astype(ml_dtypes.bfloat16)
    return np.ascontiguousarray(hi), np.ascontiguousarray(lo)


def _chunk_vec(v, kc):
    """[kc*128] -> [128, kc] with elem [p, c] = v[c*128 + p]."""
    return np.ascontiguousarray(np.asarray(v, np.float32).reshape(kc, P).T)


def _vec_pair(v, kc):
    hi, lo = _bf16_split(np.asarray(v, np.float32).reshape(-1))
    q = np.empty((P, kc, 2), dtype=ml_dtypes.bfloat16)
    q[:, :, 0] = np.asarray(hi).reshape(kc, P).T
    q[:, :, 1] = np.asarray(lo).reshape(kc, P).T
    return q


def _prep_core_inputs(a, mode, ins, bdb):
    f32 = np.float32
    m = {}
    gate_prec = {g: _gate_prec(mode, g) for g in range(4)}
    prec_b = _prec_b(mode)

    def put_w(name, w, prec):
        w = np.ascontiguousarray(w, dtype=f32)
        if prec == "bf16":
            hi, lo = _bf16_split(w)
            m[name + "_hi"] = hi
            m[name + "_lo"] = lo
        else:
            m[name] = w

    for g in range(4):
        put_w(f"Wx{g}", ins["Wx"][g, a], gate_prec[g])
        put_w(f"Wh{g}", ins["Wh"][g, a], gate_prec[g])
    for j in range(3):
        put_w(f"Wdbx{j}", ins["Wdbx"][j, a], prec_b)
        put_w(f"Wdbh{j}", ins["Wdbh"][j, a], prec_b)
    put_w("Wict", ins["W_ict"][a], prec_b)
    put_w("Wilc", ins["W_ilc"][a], prec_b)
    put_w("Wilh", ins["W_ilh"][a], prec_b)

    need_bf_vec = any(p == "bf16" for p in gate_prec.values())
    need_f32_vec = any(p == "f32" for p in gate_prec.values())
    need_f32r_vec = any(p == "f32r" for p in gate_prec.values())
    if need_bf_vec:
        m["xvq"] = _vec_pair(ins["in_"], KC_H)
        m["hvq"] = _vec_pair(ins["h0"][a], KC_H)
    if need_f32_vec:
        m["xvf"] = _chunk_vec(ins["in_"], KC_H)
        m["hvf"] = _chunk_vec(ins["h0"][a], KC_H)
    if need_f32r_vec:
        m["xvr"] = _chunk_vec(ins["in_"], KC_H)
        m["hvr"] = _chunk_vec(ins["h0"][a], KC_H)
    if prec_b == "bf16":
        m["dvq"] = _vec_pair(ins["hdb0"], KC_A)
    elif prec_b == "f32r":
        m["dvr"] = _chunk_vec(ins["hdb0"], KC_A)
    else:
        m["dvf"] = _chunk_vec(ins["hdb0"], KC_A)

    m["c0r"] = np.ascontiguousarray(ins["c0"][a], dtype=f32).reshape(1, HID)
    gb = np.empty((P, 4, KC_H), dtype=f32)
    gb[:, 0] = _chunk_vec(ins["ln_gamma"][0, a], KC_H)
    gb[:, 1] = _chunk_vec(ins["ln_beta"][0, a], KC_H)
    gb[:, 2] = _chunk_vec(ins["ln_gamma"][1, a], KC_H)
    gb[:, 3] = _chunk_vec(ins["ln_beta"][1, a], KC_H)
    m["gbc"] = gb
    m["bdbr"] = np.ascontiguousarray(bdb[:, a], dtype=f32).reshape(1, 3, AHID)
    m["bilcr"] = np.ascontiguousarray(ins["b_ilc"][a], dtype=f32).reshape(1, AHID)
    m["bilhr"] = np.ascontiguousarray(ins["b_ilh"][a], dtype=f32).reshape(1, AHID)
    return m


def _run(inputs, mode=None, **run_kwargs):
    mode = mode or MODE
    f32 = np.float32
    ins = {k: np.asarray(v) for k, v in inputs.items()}
    bdb = ins["bdbx"].astype(f32) + ins["bdbh"].astype(f32)

    in_maps = [_prep_core_inputs(a, mode, ins, bdb) for a in range(NAXIS)]

    nc = _get_nc(mode)
    res = run_bass_kernel_spmd(nc, in_maps, core_ids=list(range(NAXIS)), **run_kwargs)

    h_m = np.stack([res.results[a]["hm"].reshape(HID) for a in range(NAXIS)])
    c_m = np.stack([res.results[a]["cm"].reshape(HID) for a in range(NAXIS)])
    t1 = np.stack([res.results[a]["t1"].reshape(AHID) for a in range(NAXIS)])
    t2 = np.stack([res.results[a]["t2"].reshape(AHID) for a in range(NAXIS)])

    su_c = t1.astype(np.float64).sum(0)
    su_h = t2.astype(np.float64).sum(0)
    h_db = (1.0 / (1.0 + np.exp(-su_h))) * np.tanh(su_c)

    return (h_m.astype(f32), c_m.astype(f32), h_db.astype(f32)), res


def kernel(**inputs):
    out, _ = _run(inputs)
    return out


# revision 12
# speedup vs baseline: 1.2617x; 1.0349x over previous
"""Trainium2 Bass kernel for nn_Branch_Cell (branched LSTM-style cell).

Sharding: expert parallelism over the naxis dimension (naxis == 8 == n_cores).
Core `a` owns axis `a`: it streams that axis's ~189 MB of weights from HBM
(the memory roofline), computes h_m[a], c_m[a] and the partial sums
t1[a], t2[a]; the host sums t1/t2 over axes and applies the final
sigmoid*tanh (8 KB of work, avoids the collective latency floor).

Precision modes:
  'hybrid' (default): weights stream as host-decomposed bf16 hi + bf16 lo
     pairs (same total bytes as fp32; PE runs both halves at full bf16 rate
     with the vector's (hi, lo) pair as a [K, 2] stationary operand) --
     EXCEPT gate 1, whose output feeds a division (1 - |(xg1-hg1)/xg1|)
     that amplifies error near xg1 ~ 0; that gate streams exact fp32.
  'bf16x2': everything bf16 hi/lo.
  'f32': everything fp32 (PE at 4 cyc/row, slightly under HBM rate).

All matvecs keep the vector as the PE stationary operand (weights are the
moving operand), so kernel time is bounded by HBM->SBUF DMA. Weight blocks
are [512 rows x colspan cols] laid out [128p, 4kc, colspan] so every DMA
descriptor moves a 2 KB contiguous run (~95% of DMA line rate).
Elementwise gate/LN math runs on single-partition rows (ACT + DVE);
row->partition-chunk conversion for downstream lhsT operands uses tiny
K=1 transpose matmuls on the PE (no DMA scatters).
"""

import numpy as np
import ml_dtypes
from contextlib import ExitStack

import concourse.bass as bass
import concourse.tile as tile
from concourse import mybir
from concourse.bass_utils import run_bass_kernel_spmd

F32 = mybir.dt.float32
F32R = mybir.dt.float32r
BF16 = mybir.dt.bfloat16
AF = mybir.ActivationFunctionType
ALU = mybir.AluOpType

P = 128
INP = 2048
HID = 2048
AHID = 1024
NAXIS = 8
NT = 512                 # matmul moving free dim
KC_H = HID // P          # 16
KC_A = AHID // P         # 8

MODE = "hybrid"          # "hybrid" | "hybrid_r" | "f32r" | "bf16x2" | "f32"

# Weight dtypes per precision name
_WDT = None  # set below after mybir import resolution

import os as _os
if _os.environ.get("LDWOPT", "0") == "1":
    # walrus's own default is --enable-ldw-opt=true; bass pins it false.
    # Allow re-enabling for A/B (validated against the f64 oracle).
    import concourse.bass_utils as _bu
    _orig_run_command = _bu.run_command
    def _run_command_ldwopt(argv, **kw):
        argv = ["--enable-ldw-opt=true" if a == "--enable-ldw-opt=false" else a
                for a in argv]
        return _orig_run_command(argv, **kw)
    _bu.run_command = _run_command_ldwopt

EPS = 1e-5


# ---------------------------------------------------------------------------
# Workaround: the nix walrus in this container allows only ONE sync wait per
# non-EventSemaphore instruction ("Too many sync wait commands" in
# setupSyncWait). Tile's kernel-tail drain (and occasionally other insts)
# carries several. Split the extras onto single-wait NoOps placed just before
# the instruction on the same engine (per-engine program order preserved).
# ---------------------------------------------------------------------------
def _split_multi_waits(nc):
    n_new = 0
    for f in nc.m.functions:
        for blk in f.blocks:
            out = []
            for inst in blk.instructions:
                si = inst.sync_info
                waits = list(si.on_wait) if si is not None else []
                if len(waits) > 1 and inst.opcode != "EventSemaphore":
                    for w in waits[:-1]:
                        n_new += 1
                        out.append(
                            mybir.InstNoOp(
                                name=f"{inst.name}-wsplit{n_new}",
                                engine=inst.engine,
                                debug=inst.debug,
                                ins=[],
                                outs=[],
                                sync_info=mybir.SyncInfo(on_update=[], on_wait=[w]),
                            )
                        )
                    si.on_wait = [waits[-1]]
                    inst.sync_info = si
                out.append(inst)
            blk.instructions = out
    return n_new


def _gate_prec(mode, g):
    if mode == "f32":
        return "f32"
    if mode == "f32r":
        return "f32r"
    if mode in ("hybrid", "hybrid_r") and g == 1:
        return "f32"
    return "f32r" if mode == "hybrid_r" else "bf16"


def _prec_b(mode):
    if mode == "f32":
        return "f32"
    if mode in ("f32r", "hybrid_r"):
        return "f32r"
    return "bf16"


# ---------------------------------------------------------------------------
# Kernel builder
# ---------------------------------------------------------------------------
def _build(mode):
    nc = bass.Bass()
    prec_b = _prec_b(mode)

    def din(name, shape, dt=F32):
        return nc.dram_tensor(name, shape, dt, kind="ExternalInput")

    def dout(name, shape, dt=F32):
        return nc.dram_tensor(name, shape, dt, kind="ExternalOutput")

    def w_pair(name, shape, prec):
        """Declare weight tensor(s): (hi_handle, lo_handle|None)."""
        if prec == "bf16":
            return din(name + "_hi", shape, BF16), din(name + "_lo", shape, BF16)
        return din(name, shape, F32 if prec == "f32" else F32R), None

    gate_prec = {g: _gate_prec(mode, g) for g in range(4)}
    Wx_t = {g: w_pair(f"Wx{g}", [INP, HID], gate_prec[g]) for g in range(4)}
    Wh_t = {g: w_pair(f"Wh{g}", [HID, HID], gate_prec[g]) for g in range(4)}
    Wdbx_t = {j: w_pair(f"Wdbx{j}", [HID, AHID], prec_b) for j in range(3)}
    Wdbh_t = {j: w_pair(f"Wdbh{j}", [AHID, AHID], prec_b) for j in range(3)}
    Wict_t = w_pair("Wict", [HID, AHID], prec_b)
    Wilc_t = w_pair("Wilc", [AHID, AHID], prec_b)
    Wilh_t = w_pair("Wilh", [AHID, AHID], prec_b)

    need_bf_vec = any(p == "bf16" for p in gate_prec.values())
    need_f32_vec = any(p == "f32" for p in gate_prec.values())
    need_f32r_vec = any(p == "f32r" for p in gate_prec.values())

    xv_t = din("xvq", [P, KC_H, 2], BF16) if need_bf_vec else None
    hv_t = din("hvq", [P, KC_H, 2], BF16) if need_bf_vec else None
    xvf_t = din("xvf", [P, KC_H], F32) if need_f32_vec else None
    hvf_t = din("hvf", [P, KC_H], F32) if need_f32_vec else None
    xvr_t = din("xvr", [P, KC_H], F32R) if need_f32r_vec else None
    hvr_t = din("hvr", [P, KC_H], F32R) if need_f32r_vec else None
    if prec_b == "bf16":
        dv_t = din("dvq", [P, KC_A, 2], BF16)
    elif prec_b == "f32r":
        dv_t = din("dvr", [P, KC_A], F32R)
    else:
        dv_t = din("dvf", [P, KC_A], F32)

    c0_t = din("c0r", [1, HID])
    gb_t = din("gbc", [P, 4, KC_H])      # g0,b0,g1,b1 chunked [p, i, kc]
    bdb_t = din("bdbr", [1, 3, AHID])    # bdbx[:,a] + bdbh[:,a]
    bilc_t = din("bilcr", [1, AHID])
    bilh_t = din("bilhr", [1, AHID])

    hm_t = dout("hm", [1, HID])
    cm_t = dout("cm", [1, HID])
    t1_t = dout("t1", [1, AHID])
    t2_t = dout("t2", [1, AHID])

    with tile.TileContext(nc) as tc:
        with ExitStack() as ctx:
            wpool = ctx.enter_context(tc.tile_pool(name="wstream", bufs=6))
            rows = ctx.enter_context(tc.tile_pool(name="rows", bufs=1))
            stmps = ctx.enter_context(tc.tile_pool(name="stmps", bufs=3))
            kcp = ctx.enter_context(tc.tile_pool(name="kcp", bufs=1))
            sm = ctx.enter_context(tc.tile_pool(name="smalls", bufs=1))
            pmm = ctx.enter_context(tc.tile_pool(name="pmm", bufs=4, space="PSUM"))
            ptr = ctx.enter_context(tc.tile_pool(name="ptr", bufs=2, space="PSUM"))
            pcol = ctx.enter_context(tc.tile_pool(name="pcol", bufs=2, space="PSUM"))

            # ---- small input loads -----------------------------------
            xv = hv = xvf = hvf = xvr = hvr = None
            if need_bf_vec:
                xv = kcp.tile([P, KC_H, 2], BF16, tag="xv")
                hv = kcp.tile([P, KC_H, 2], BF16, tag="hv")
                nc.sync.dma_start(xv[:], xv_t[:])
                nc.sync.dma_start(hv[:], hv_t[:])
            if need_f32_vec:
                xvf = kcp.tile([P, KC_H], F32, tag="xvf")
                hvf = kcp.tile([P, KC_H], F32, tag="hvf")
                nc.sync.dma_start(xvf[:], xvf_t[:])
                nc.sync.dma_start(hvf[:], hvf_t[:])
            if need_f32r_vec:
                xvr = kcp.tile([P, KC_H], F32R, tag="xvr")
                hvr = kcp.tile([P, KC_H], F32R, tag="hvr")
                nc.sync.dma_start(xvr[:], xvr_t[:])
                nc.sync.dma_start(hvr[:], hvr_t[:])
            if prec_b == "bf16":
                dv = kcp.tile([P, KC_A, 2], BF16, tag="dv")
            elif prec_b == "f32r":
                dv = kcp.tile([P, KC_A], F32R, tag="dv")
            else:
                dv = kcp.tile([P, KC_A], F32, tag="dv")
            nc.sync.dma_start(dv[:], dv_t[:])

            ones = sm.tile([1, 1], F32, tag="ones")
            nc.vector.memset(ones[:], 1.0)
            ones2 = sm.tile([2, 1], F32, tag="ones2")
            nc.vector.memset(ones2[:], 1.0)

            # ---- helpers ---------------------------------------------
            def collapse(ps, prec):
                """psum [M, 512] -> psum row [1, 512] AP (hi+lo summed).

                PSUM reads must start at partition 0 and a DVE op may read
                only one PSUM operand, so the two rows are combined on the
                PE: copy [2, NT] to SBUF, then ones2.T @ t sums the rows.
                """
                if prec != "bf16":
                    return ps[0:1, :]
                t = stmps.tile([2, NT], F32, tag="clp")
                nc.vector.tensor_copy(t[:], ps[:])
                pc = pcol.tile([1, NT], F32, tag="pcol")
                nc.tensor.matmul(pc[:], ones2[0:2, 0:1], t[:], start=True, stop=True)
                return pc

            # Consume-stage software pipeline: each column-block's psum
            # collapse/activation is deferred until the NEXT block's matmuls
            # have been emitted, so the in-order PE never stalls on the DVE
            # copy feeding the collapse matmul.
            deferred = []

            def flush_deferred():
                while deferred:
                    deferred.pop(0)()

            def stream_group(contribs, w_cols, prec, consume, scope):
                """Sum_i vec_i.T @ W_i over full K, streamed in row-contiguous
                blocks; calls consume(col_off, ps, prec) per 512-col output
                (deferred by one block).

                contribs: list of (vec_tile, (w_hi, w_lo), K).
                Weight blocks are [512, colspan] -> [128, 4, colspan] tiles
                (every DMA descriptor run = colspan * dtype = 2 KB).
                """
                colspan = 1024 if prec == "bf16" else 512
                n_ct = colspan // NT
                m_dim = 2 if prec == "bf16" else 1
                n_halves = 2 if prec == "bf16" else 1
                wdt = {"bf16": BF16, "f32": F32, "f32r": F32R}[prec]
                wtag = "wblk"
                wbufs = 8
                total_per_ct = sum((K // NT) * 4 * n_halves for _, _, K in contribs)

                for cb in range(w_cols // colspan):
                    with nc.named_scope(f"{scope}_cb{cb}"):
                        pss = [
                            pmm.tile([m_dim, NT], F32, tag="mmps", name="mmps")
                            for _ in range(n_ct)
                        ]
                        cnt = 0
                        n_blk = 0
                        for vec, (w_hi, w_lo), K in contribs:
                            for kb in range(K // NT):
                                for w_t in (w_hi, w_lo)[:n_halves]:
                                    blk = wpool.tile(
                                        [P, 4, colspan], wdt, tag=wtag, bufs=wbufs
                                    )
                                    nc.sync.dma_start(
                                        blk[:],
                                        w_t[
                                            kb * NT : (kb + 1) * NT,
                                            cb * colspan : (cb + 1) * colspan,
                                        ].rearrange("(kc p) n -> p kc n", p=P),
                                    )
                                    for kc in range(4):
                                        ki = kb * 4 + kc
                                        lhsT = (
                                            vec[:, ki, :]
                                            if prec == "bf16"
                                            else vec[:, ki : ki + 1]
                                        )
                                        for ct in range(n_ct):
                                            nc.tensor.matmul(
                                                pss[ct][:],
                                                lhsT,
                                                blk[:, kc, ct * NT : (ct + 1) * NT],
                                                start=(cnt // n_ct == 0),
                                                stop=(cnt // n_ct == total_per_ct - 1),
                                            )
                                            cnt += 1
                                    n_blk += 1
                                    if n_blk == 1:
                                        flush_deferred()

                        def _consume_cb(cb=cb, pss=pss):
                            for ct in range(n_ct):
                                consume(cb * colspan + ct * NT, pss[ct], prec)

                        deferred.append(_consume_cb)

            def pe_transpose(row_ap, cpt):
                """[1, cpt*128] f32 row -> psum tile [P, cpt] f32."""
                pst = ptr.tile([P, KC_H], F32, tag="ptr")
                for c in range(cpt):
                    nc.tensor.matmul(
                        pst[:, c : c + 1],
                        row_ap[0:1, c * P : (c + 1) * P],
                        ones[0:1, 0:1],
                        start=True,
                        stop=True,
                    )
                return pst

            def to_lhsT(src_ap, cpt, name):
                """[P, cpt] f32 (psum/sbuf) -> lhsT tile for stage-B matvecs."""
                if prec_b == "bf16":
                    tq = kcp.tile([P, cpt, 2], BF16, tag=name)
                    nc.vector.tensor_copy(tq[:, :, 0], src_ap)
                    hi32 = kcp.tile([P, cpt], F32, tag=name + "32")
                    nc.vector.tensor_copy(hi32[:], tq[:, :, 0])
                    nc.vector.tensor_sub(tq[:, :, 1], src_ap, hi32[:])
                    return tq
                wdt = F32R if prec_b == "f32r" else F32
                tq = kcp.tile([P, cpt], wdt, tag=name)
                nc.vector.tensor_copy(tq[:], src_ap)
                return tq

            def layernorm_lhsT(src_row, gb_idx, name):
                """LN over [1, HID] row; gamma/beta applied after the
                row->[P, KC_H] transpose. Returns stage-B lhsT tile."""
                stats = sm.tile([1, HID // 512, 6], F32, tag=name + "st")
                for c_ in range(HID // 512):
                    nc.vector.bn_stats(
                        stats[:, c_, :], src_row[:, c_ * 512 : (c_ + 1) * 512]
                    )
                mv = sm.tile([1, 2], F32, tag=name + "mv")
                nc.vector.bn_aggr(mv[:], stats[:])
                vs = sm.tile([1, 1], F32, tag=name + "vs")
                nc.vector.tensor_scalar_add(vs[:], mv[:, 1:2], EPS)
                sd = sm.tile([1, 1], F32, tag=name + "sd")
                nc.scalar.sqrt(sd[:], vs[:])
                inv = sm.tile([1, 1], F32, tag=name + "inv")
                nc.vector.reciprocal(inv[:], sd[:])
                nmu = sm.tile([1, 1], F32, tag=name + "nmu")
                nc.vector.scalar_tensor_tensor(
                    nmu[:], mv[:, 0:1], -1.0, inv[:], ALU.mult, ALU.mult
                )
                ln0 = rows.tile([1, HID], F32, tag="ln0")
                nc.scalar.activation(
                    ln0[:], src_row[:], AF.Identity, bias=nmu[:], scale=inv[:]
                )
                pst = pe_transpose(ln0, KC_H)
                lnc = kcp.tile([P, KC_H], F32, tag=name + "c")
                nc.vector.tensor_mul(lnc[:], pst[:, :KC_H], gbc[:, gb_idx, :])
                nc.vector.tensor_add(lnc[:], lnc[:], gbc[:, gb_idx + 1, :])
                return to_lhsT(lnc[:], KC_H, name + "kc")

            def gvec(prec):
                if prec == "bf16":
                    return (xv, hv)
                if prec == "f32r":
                    return (xvr, hvr)
                return (xvf, hvf)

            # ---- stage A: gates --------------------------------------
            ig_r = rows.tile([1, HID], F32, tag="ig")
            gg_r = rows.tile([1, HID], F32, tag="gg")
            og_r = rows.tile([1, HID], F32, tag="og")
            d_r = rows.tile([1, HID], F32, tag="dr")
            rc_r = rows.tile([1, HID], F32, tag="rc")

            gate_rows = {0: ig_r, 2: gg_r, 3: og_r}
            gate_fn = {0: AF.Sigmoid, 2: AF.Tanh, 3: AF.Sigmoid}

            def emit_gate(g):
                pg = gate_prec[g]
                xvec, hvec = gvec(pg)

                def mk_consume(gr=gate_rows[g], fn=gate_fn[g]):
                    def consume(col, ps, prec):
                        nc.scalar.activation(
                            gr[:, col : col + NT], collapse(ps, prec), fn
                        )
                    return consume

                stream_group(
                    [(xvec, Wx_t[g], INP), (hvec, Wh_t[g], HID)],
                    HID, pg, mk_consume(), f"g{g}",
                )

            def consume_x(col, ps, prec):
                pc = collapse(ps, prec)
                nc.vector.tensor_copy(d_r[:, col : col + NT], pc)
                nc.vector.reciprocal(rc_r[:, col : col + NT], pc)

            def consume_h(col, ps, prec):
                nc.vector.tensor_sub(
                    d_r[:, col : col + NT],
                    d_r[:, col : col + NT],
                    collapse(ps, prec),
                )

            pg1 = gate_prec[1]
            xv1, hv1 = gvec(pg1)
            # interleave the (PE-heavier) fp32 gate-1 passes between the
            # bf16 gates so the PE hump is absorbed by the DMA prefetch bufs
            emit_gate(0)
            # small loads not needed until much later: emitted here so their
            # DMA-queue slots come after the first weight blocks (faster start)
            c0r = rows.tile([1, HID], F32, tag="c0r")
            nc.sync.dma_start(c0r[:], c0_t[:])
            gbc = kcp.tile([P, 4, KC_H], F32, tag="gbc")
            nc.sync.dma_start(gbc[:], gb_t[:])
            bdbr = rows.tile([1, 3, AHID], F32, tag="bdbr")
            nc.sync.dma_start(bdbr[:], bdb_t[:])
            bilcr = rows.tile([1, AHID], F32, tag="bilcr")
            nc.sync.dma_start(bilcr[:], bilc_t[:])
            bilhr = rows.tile([1, AHID], F32, tag="bilhr")
            nc.sync.dma_start(bilhr[:], bilh_t[:])
            stream_group([(xv1, Wx_t[1], INP)], HID, pg1, consume_x, "g1x")
            emit_gate(2)
            stream_group([(hv1, Wh_t[1], HID)], HID, pg1, consume_h, "g1h")
            emit_gate(3)

            # ---- stage A chain: c_m, h_m, layernorms ------------------
            flush_deferred()
            with nc.named_scope("gate_chain"):
                # q = d / xg1   (in place of d)
                nc.vector.tensor_mul(d_r[:], d_r[:], rc_r[:])
                # |q|  (in place of rc)
                nc.scalar.activation(rc_r[:], d_r[:], AF.Abs)
                # |q| * c0   (into d: q dead)
                nc.vector.tensor_mul(d_r[:], rc_r[:], c0r[:])
                # ig * gg  (in place of ig)
                nc.vector.tensor_mul(ig_r[:], ig_r[:], gg_r[:])
                # fg_p*c0 = c0 - |q|*c0   (into rc: |q| dead)
                nc.vector.tensor_sub(rc_r[:], c0r[:], d_r[:])
                cm_r = rows.tile([1, HID], F32, tag="cmr")
                nc.vector.tensor_add(cm_r[:], rc_r[:], ig_r[:])
                nc.sync.dma_start(cm_t[:], cm_r[:])

            with nc.named_scope("ln_c"):
                lc_kc = layernorm_lhsT(cm_r, 2, "lnc")

            with nc.named_scope("h_m"):
                th_r = rows.tile([1, HID], F32, tag="gg")  # reuse gg slot
                nc.scalar.activation(th_r[:], cm_r[:], AF.Tanh)
                # h_m = og * tanh(c_m)  (in place of og)
                nc.vector.tensor_mul(og_r[:], og_r[:], th_r[:])
                nc.sync.dma_start(hm_t[:], og_r[:])

            with nc.named_scope("ln_h"):
                ht_kc = layernorm_lhsT(og_r, 0, "lnh")

            # ---- stage B ---------------------------------------------
            ct_r = rows.tile([1, AHID], F32, tag="ctr")

            def consume_ct(col, ps, prec):
                nc.vector.tensor_copy(ct_r[:, col : col + NT], collapse(ps, prec))

            stream_group([(lc_kc, Wict_t, HID)], AHID, prec_b, consume_ct, "ict")

            indb_r = rows.tile([1, AHID], F32, tag="indb")
            fndb_r = rows.tile([1, AHID], F32, tag="fndb")
            cndb_r = rows.tile([1, AHID], F32, tag="cndb")
            pre_rows = {0: indb_r, 1: fndb_r, 2: cndb_r}
            pre_fn = {0: AF.Sigmoid, 1: AF.Sigmoid, 2: AF.Tanh}

            def emit_db(j):
                def mk_consume_pre(jj=j):
                    def consume(col, ps, prec):
                        srow = stmps.tile([1, NT], F32, tag="stmp2")
                        nc.vector.tensor_add(
                            srow[:], collapse(ps, prec),
                            bdbr[:, jj, col : col + NT],
                        )
                        nc.scalar.activation(
                            pre_rows[jj][:, col : col + NT], srow[:], pre_fn[jj]
                        )
                    return consume

                stream_group(
                    [(ht_kc, Wdbx_t[j], HID), (dv, Wdbh_t[j], AHID)],
                    AHID, prec_b, mk_consume_pre(), f"db{j}",
                )

            def mk_consume_il(bias_r, out_t):
                def consume(col, ps, prec):
                    srow = stmps.tile([1, NT], F32, tag="stmp3")
                    nc.vector.tensor_add(
                        srow[:], collapse(ps, prec), bias_r[:, col : col + NT]
                    )
                    nc.sync.dma_start(out_t[:, col : col + NT], srow[:])
                return consume

            # ---- order: db0 -> i_cell/t1 overlap db1+db2 -> c_cell/t2 ----
            emit_db(0)
            flush_deferred()
            with nc.named_scope("cells1"):
                # i_cell = in_db * c_t (in place of indb)
                nc.vector.tensor_mul(indb_r[:], indb_r[:], ct_r[:])
                ic_kc = to_lhsT(pe_transpose(indb_r, KC_A)[:, :KC_A], KC_A, "ickc")
            stream_group(
                [(ic_kc, Wilc_t, AHID)], AHID, prec_b,
                mk_consume_il(bilcr, t1_t), "t1",
            )
            emit_db(1)
            emit_db(2)
            flush_deferred()
            with nc.named_scope("cells2"):
                # fn_db * cn_db (in place of fndb)
                nc.vector.tensor_mul(fndb_r[:], fndb_r[:], cndb_r[:])
                # c_cell (in place of cndb)
                nc.vector.tensor_add(cndb_r[:], fndb_r[:], indb_r[:])
                cc_kc = to_lhsT(pe_transpose(cndb_r, KC_A)[:, :KC_A], KC_A, "cckc")
            stream_group(
                [(cc_kc, Wilh_t, AHID)], AHID, prec_b,
                mk_consume_il(bilhr, t2_t), "t2",
            )
            flush_deferred()

    _split_multi_waits(nc)
    return nc


_NC_CACHE = {}


def _get_nc(mode):
    if mode not in _NC_CACHE:
        _NC_CACHE[mode] = _build(mode)
    return _NC_CACHE[mode]


# ---------------------------------------------------------------------------
# Host side: shard, run, gather
# ---------------------------------------------------------------------------
def _bf16_split(w):
    """fp32 array -> (hi, lo) bf16 arrays with hi + lo ~= w (round-to-nearest).

    Bit-twiddled for speed: hi = RNE-round to bf16; lo = RNE(w - hi).
    """
    w = np.ascontiguousarray(w, dtype=np.float32)
    u = w.view(np.uint32)
    rhi = (u + 0x7FFF + ((u >> 16) & 1)) & 0xFFFF0000
    hi32 = rhi.view(np.float32)
    hi = (rhi >> 16).astype(np.uint16).view(ml_dtypes.bfloat16)
    lo = (w - hi32).astype(ml_dtypes.bfloat16)
    return np.ascontiguousarray(hi), np.ascontiguousarray(lo)


def _chunk_vec(v, kc):
    """[kc*128] -> [128, kc] with elem [p, c] = v[c*128 + p]."""
    return np.ascontiguousarray(np.asarray(v, np.float32).reshape(kc, P).T)


def _vec_pair(v, kc):
    hi, lo = _bf16_split(np.asarray(v, np.float32).reshape(-1))
    q = np.empty((P, kc, 2), dtype=ml_dtypes.bfloat16)
    q[:, :, 0] = np.asarray(hi).reshape(kc, P).T
    q[:, :, 1] = np.asarray(lo).reshape(kc, P).T
    return q


def _prep_core_inputs(a, mode, ins, bdb):
    f32 = np.float32
    m = {}
    gate_prec = {g: _gate_prec(mode, g) for g in range(4)}
    prec_b = _prec_b(mode)

    def put_w(name, w, prec):
        w = np.ascontiguousarray(w, dtype=f32)
        if prec == "bf16":
            hi, lo = _bf16_split(w)
            m[name + "_hi"] = hi
            m[name + "_lo"] = lo
        else:
            m[name] = w

    for g in range(4):
        put_w(f"Wx{g}", ins["Wx"][g, a], gate_prec[g])
        put_w(f"Wh{g}", ins["Wh"][g, a], gate_prec[g])
    for j in range(3):
        put_w(f"Wdbx{j}", ins["Wdbx"][j, a], prec_b)
        put_w(f"Wdbh{j}", ins["Wdbh"][j, a], prec_b)
    put_w("Wict", ins["W_ict"][a], prec_b)
    put_w("Wilc", ins["W_ilc"][a], prec_b)
    put_w("Wilh", ins["W_ilh"][a], prec_b)

    need_bf_vec = any(p == "bf16" for p in gate_prec.values())
    need_f32_vec = any(p == "f32" for p in gate_prec.values())
    need_f32r_vec = any(p == "f32r" for p in gate_prec.values())
    if need_bf_vec:
        m["xvq"] = _vec_pair(ins["in_"], KC_H)
        m["hvq"] = _vec_pair(ins["h0"][a], KC_H)
    if need_f32_vec:
        m["xvf"] = _chunk_vec(ins["in_"], KC_H)
        m["hvf"] = _chunk_vec(ins["h0"][a], KC_H)
    if need_f32r_vec:
        m["xvr"] = _chunk_vec(ins["in_"], KC_H)
        m["hvr"] = _chunk_vec(ins["h0"][a], KC_H)
    if prec_b == "bf16":
        m["dvq"] = _vec_pair(ins["hdb0"], KC_A)
    elif prec_b == "f32r":
        m["dvr"] = _chunk_vec(ins["hdb0"], KC_A)
    else:
        m["dvf"] = _chunk_vec(ins["hdb0"], KC_A)

    m["c0r"] = np.ascontiguousarray(ins["c0"][a], dtype=f32).reshape(1, HID)
    gb = np.empty((P, 4, KC_H), dtype=f32)
    gb[:, 0] = _chunk_vec(ins["ln_gamma"][0, a], KC_H)
    gb[:, 1] = _chunk_vec(ins["ln_beta"][0, a], KC_H)
    gb[:, 2] = _chunk_vec(ins["ln_gamma"][1, a], KC_H)
    gb[:, 3] = _chunk_vec(ins["ln_beta"][1, a], KC_H)
    m["gbc"] = gb
    m["bdbr"] = np.ascontiguousarray(bdb[:, a], dtype=f32).reshape(1, 3, AHID)
    m["bilcr"] = np.ascontiguousarray(ins["b_ilc"][a], dtype=f32).reshape(1, AHID)
    m["bilhr"] = np.ascontiguousarray(ins["b_ilh"][a], dtype=f32).reshape(1, AHID)
    return m


def _run(inputs, mode=None, **run_kwargs):
    mode = mode or MODE
    f32 = np.float32
    ins = {k: np.asarray(v) for k, v in inputs.items()}
    bdb = ins["bdbx"].astype(f32) + ins["bdbh"].astype(f32)

    in_maps = [_prep_core_inputs(a, mode, ins, bdb) for a in range(NAXIS)]

    nc = _get_nc(mode)
    res = run_bass_kernel_spmd(nc, in_maps, core_ids=list(range(NAXIS)), **run_kwargs)

    h_m = np.stack([res.results[a]["hm"].reshape(HID) for a in range(NAXIS)])
    c_m = np.stack([res.results[a]["cm"].reshape(HID) for a in range(NAXIS)])
    t1 = np.stack([res.results[a]["t1"].reshape(AHID) for a in range(NAXIS)])
    t2 = np.stack([res.results[a]["t2"].reshape(AHID) for a in range(NAXIS)])

    su_c = t1.astype(np.float64).sum(0)
    su_h = t2.astype(np.float64).sum(0)
    h_db = (1.0 / (1.0 + np.exp(-su_h))) * np.tanh(su_c)

    return (h_m.astype(f32), c_m.astype(f32), h_db.astype(f32)), res


def kernel(**inputs):
    out, _ = _run(inputs)
    return out
